# revision 58
# baseline (speedup 1.0000x reference)
# Bloom parallel attention block on 8 trn2 NeuronCores, tensor-parallel over
# heads (2 heads per core).  Feature-major layouts, fp8e4 datapath
# (residual/bias/psum fp32; exp intermediates bf16).
#
# Per core r (heads 2r, 2r+1):
#   QKV matmul in fp8e4 DoubleRow (2 h-tiles per PE op, ~2x bf16/instr):
#     hid fp8, weights host-scaled x64 (fp8 subnormal escape), descaled on
#     the DVE evacuation via dual-scalar tensor_scalar (psum*(1/64)+bias).
#     -> Q^T/K^T [d, s] fp8 and V^T [d, s] bf16 per batch in SBUF.
#     inv_norm (1/sqrt(hd)) is applied in the ACT Exp scale, not the weights.
#   V^T is transposed on the PE to V [s, d] and scaled by exp(alibi[k]) on
#   evacuation (fp8); the softmax-denominator matmul weights are
#   exp(alibi[k])/16 broadcast columns ("ones'", fp8, built on gpsimd).
#   This folds alibi in MULTIPLICATIVELY:
#     exp(s + a) * mask = exp(s) * mask * exp(a)
#   The /16 makes rec = 16/sum so ctx^T*rec lands at fp8-friendly scale.
#   attention (per b, head hl, 512-wide q-chunk qc), scores transposed [k, q]:
#     scores^T = K^T_tile.T @ Q^T       fp8, fp32 psum, per k-tile
#     exp(inv_norm*scores - 3.5)        (ACT, 2 k-tiles/op, bf16 out; the
#                                        mask carries e^-2 more: max score
#                                        is 10.24 and fp8 has no saturation)
#     * mask01^T                        (DVE, [P,4,512] op, -> fp8 probs)
#     ctx^T += V'_pair.T @ prob_pair    (PE fp8 DoubleRow, 8 ops/16 k-tiles)
#     sum   += ones'_pair.T @ prob_pair (PE fp8 DoubleRow, denominator)
#     ctx^T *= 16/sum -> fp8 -> DMA to cc chunk
#   Pipelining: QKV(b1) matmuls are interleaved into attention(b0) k-loops,
#   dense matmuls into attention(b1) k-loops, so the PE never idles.  The
#   mask stays fully resident (shared by both batches); the sync queue
#   carries only wq/hid/ctxn/dctx/out, gpsimd carries collectives + cold
#   loads (HWDGE DMAs occupy their queue for the whole transfer).  ctx is
#   AllGathered in fp8: b0 in 2 column chunks, b1 in 8 per-(qc,head)
#   chunks so the final gather is small and the tail drains fast.
#   dense: fp8 DoubleRow (wd host-scaled x64; ctx fp8 carries x16; wdT2 is
#     row-permuted even/odd to match the per-head b1 gather layout):
#     out^T[o_local, s] = wdT_tile.T @ ctx^T_full; evac = ACT Copy
#     (psum/1024) + DVE add of (residual^T + b_dense).
#     (column-parallel => no all-reduce; host concatenates output slices)
import os
import sys

import numpy as np

if "/opt/trn_rl_repo" not in sys.path:
    sys.path.insert(0, "/opt/trn_rl_repo")

import ml_dtypes

import concourse.bass as bass
import concourse.mybir as mybir
import concourse.tile as tile
from concourse import bacc, bass_utils

B, S, H, NH = 2, 2048, 2048, 16
HD = H // NH            # 128
NCORES = 8
HPC = NH // NCORES      # heads per core = 2
OSH = 3 * H // NCORES   # qkv output rows per core = 768
DSH = H // NCORES       # dense output cols per core = 256
P = 128
F32 = mybir.dt.float32
BF16 = mybir.dt.bfloat16
FP8 = mybir.dt.float8e4
AF = mybir.ActivationFunctionType
ALU = mybir.AluOpType
DR = mybir.MatmulPerfMode.DoubleRow
NPBF16 = ml_dtypes.bfloat16
NPFP8 = ml_dtypes.float8_e4m3

WSCALE = 64.0           # host scale on wq/wd to escape fp8 subnormals
OWSCALE = 1.0 / 16.0    # ones' scale => ctx fp8 carries x16
INV_NORM = 1.0 / np.sqrt(HD)


def build_nc():
    nc = bacc.Bacc(
        "TRN2",
        target_bir_lowering=False,
        debug=False,
        num_devices=NCORES,
    )

    # hidT packed per 512-wide s-chunk (contiguous 1MB per chunk => long DMA
    # descriptors); mask01T packed per q-chunk likewise
    hidT = nc.dram_tensor("hidT", [B * 4, H, 512], FP8, kind="ExternalInput").ap()
    wqkvT = nc.dram_tensor("wqkvT", [H, OSH], FP8, kind="ExternalInput").ap()
    bqkv = nc.dram_tensor("bqkv", [P, 6], F32, kind="ExternalInput").ap()
    mask01T = nc.dram_tensor("mask01T", [4, S, 512], BF16, kind="ExternalInput").ap()
    # cols 0-63: exp(alibi) (V' scale); cols 64-127: exp(alibi)/16 (ones')
    alibi_e = nc.dram_tensor("alibi_e", [P, 4 * HPC * 16], F32, kind="ExternalInput").ap()
    # ones' tiles prebuilt on host (exp(alibi[k])/16 broadcast across 128
    # cols, fp8): building them on-device costs 64 DVE/gpsimd ops
    owT = nc.dram_tensor("owT", [P, B * HPC * 16 * P], FP8, kind="ExternalInput").ap()
    wdT = nc.dram_tensor("wdT", [H, DSH], FP8, kind="ExternalInput").ap()
    # h-tile rows permuted [0,2,..,14,1,3,..,15] for the per-head b1 gathers
    wdT2 = nc.dram_tensor("wdT2", [H, DSH], FP8, kind="ExternalInput").ap()
    residT = nc.dram_tensor("residT", [DSH, B * S], F32, kind="ExternalInput").ap()
    eye = nc.dram_tensor("eye", [P, P], BF16, kind="ExternalInput").ap()
    outT = nc.dram_tensor("outT", [DSH, B * S], F32, kind="ExternalOutput").ap()

    with tile.TileContext(nc) as tc:
        ccg = [list(range(NCORES))]
        with (
            tc.tile_pool(name="const", bufs=1) as constp,
            tc.tile_pool(name="dram", bufs=1, space="DRAM") as dramp,
        ):
            bq_sb = constp.tile([P, 6], F32)
            nc.gpsimd.dma_start(bq_sb, bqkv)
            # shared bias column for the exps: passing a float bias makes
            # bass materialize a const AP per activation call (~25us of DVE
            # setup for 128 exps).  The -3.5 shift keeps exp outputs under
            # fp8e4's 240 max normal (measured score max ~7.8 sigma, and
            # the fp8 cast does NOT saturate: overflow becomes inf); the
            # shift cancels in the ctx/sum ratio.
            shift_col = constp.tile([P, 1], F32)
            nc.vector.memset(shift_col, -3.5)
            ale_sb = constp.tile([P, 4 * HPC * 16], F32)
            nc.gpsimd.dma_start(ale_sb, alibi_e)
            eye_sb = constp.tile([P, P], BF16)
            nc.gpsimd.dma_start(eye_sb, eye)

            # ctx gather chunks (fp8): b0 in 4 per-qc chunks [2 heads x 512]
            # (gathered as soon as each q-chunk completes in phase 2),
            # b1 in 8 per-(qc, head) chunks [1 head x 512] so the last gather
            # is tiny and the pipeline drains quickly at the tail.
            cc_in0 = [
                dramp.tile([HPC * HD, S // 2], FP8, name=f"cc_in0{i}")
                for i in range(2)
            ]
            cc_out0 = [
                dramp.tile([NCORES * HPC * HD, S // 2], FP8, addr_space="Shared",
                           name=f"cc_out0{i}")
                for i in range(2)
            ]
            cc_in1 = [
                [dramp.tile([HD, 512], FP8, name=f"cc_in1{qc}{hl}") for hl in range(HPC)]
                for qc in range(4)
            ]
            cc_out1 = [
                [
                    dramp.tile([NCORES * HD, 512], FP8, addr_space="Shared",
                               name=f"cc_out1{qc}{hl}")
                    for hl in range(HPC)
                ]
                for qc in range(4)
            ]

            def dma_ctx(b, qc, hl, ctxn_t):
                if b == 0:
                    chunk, qq = divmod(qc, 2)
                    nc.sync.dma_start(
                        cc_in0[chunk][hl * P : (hl + 1) * P, qq * 512 : (qq + 1) * 512],
                        ctxn_t,
                    )
                else:
                    nc.sync.dma_start(cc_in1[qc][hl], ctxn_t)

            def all_gather0(chunk):
                nc.gpsimd.collective_compute(
                    "AllGather", ALU.bypass, replica_groups=ccg,
                    ins=[cc_in0[chunk].opt()], outs=[cc_out0[chunk].opt()],
                )

            def all_gather1(qc, hl):
                nc.gpsimd.collective_compute(
                    "AllGather", ALU.bypass, replica_groups=ccg,
                    ins=[cc_in1[qc][hl].opt()], outs=[cc_out1[qc][hl].opt()],
                )

            with (
                tc.tile_pool(name="mask", bufs=4) as maskp,
                tc.tile_pool(name="qk1", bufs=1) as qk1p,
                tc.tile_pool(name="vt", bufs=1) as vtp,
                tc.tile_pool(name="v1", bufs=1) as v1p,
                tc.tile_pool(name="ow1", bufs=1) as ow1p,
                tc.tile_pool(name="dw", bufs=1) as dwp,
                tc.tile_pool(name="att", bufs=1) as attp,
                tc.tile_pool(name="aps", bufs=1, space="PSUM") as aps,
            ):
                qk_sbs = [None, qk1p.tile([P, 2 * HPC, S], FP8, name="qksb1")]
                v_sbs = [None, v1p.tile([P, HPC, 16, P], FP8, name="vsb1")]
                ow_sbs = [None, ow1p.tile([P, HPC, 16, P], FP8, name="owsb1")]

                def load_mask(qc):
                    """All 4 mask q-chunks stay resident (the mask is shared
                    by both batches, so phases 2 and 3 reuse the same tiles
                    and NO mask DMA competes with compute).  Loaded on
                    gpsimd at phase-1 end, before any collective trigger
                    can block that queue."""
                    m = maskp.tile([P, 16, 512], BF16, tag="mask")
                    nc.gpsimd.dma_start(
                        m,
                        mask01T[qc].rearrange("(kt p) q -> p kt q", p=P),
                    )
                    return m

                def attn_block(b, qc, hl, mask_sb, aps, attp, extra_mm):
                    """Attention for (b, head hl, q-chunk qc).  k-tiles are
                    processed in groups of 4 (2 score-psum pairs, 1 wide mask
                    mul); extra_mm(kp) for kp in 0..7 emits independent
                    matmuls to keep the PE busy while ACT/DVE run."""
                    qk = qk_sbs[b]
                    ctx_ps = aps.tile([P, 512], F32, tag="ctx", bufs=1)
                    sum_ps = aps.tile([P, 512], F32, tag="sum", bufs=1)
                    for kg in range(4):
                        kt0 = 4 * kg
                        exp_t = attp.tile([P, 4, 512], BF16, tag="exp", bufs=2)
                        for u2 in range(2):
                            s_ps = aps.tile([P, 1024], F32, tag="sco", bufs=2)
                            for u in range(2):
                                kt = kt0 + 2 * u2 + u
                                nc.tensor.matmul(
                                    s_ps[:, u * 512 : (u + 1) * 512],
                                    lhsT=qk[:, hl * 2 + 1, kt * P : (kt + 1) * P],
                                    rhs=qk[:, hl * 2, qc * 512 : (qc + 1) * 512],
                                    start=True,
                                    stop=True,
                                )
                            nc.scalar.activation(
                                exp_t[:, 2 * u2 : 2 * u2 + 2, :],
                                s_ps.rearrange("p (u q) -> p u q", u=2),
                                AF.Exp,
                                bias=shift_col[:, 0:1],
                                scale=float(INV_NORM),
                            )
                        prob_t = attp.tile([P, 4, 512], FP8, tag="prob", bufs=2)
                        nc.vector.tensor_mul(
                            prob_t,
                            exp_t,
                            mask_sb[:, kt0 : kt0 + 4, :],
                        )
                        for u2 in range(2):
                            kt = kt0 + 2 * u2
                            ph = prob_t[:, 2 * u2 : 2 * u2 + 2, :]
                            nc.tensor.matmul(
                                ctx_ps,
                                lhsT=v_sbs[b][:, hl, kt : kt + 2, :],
                                rhs=ph,
                                start=(kt == 0),
                                stop=(kt == 14),
                                perf_mode=DR,
                            )
                            nc.tensor.matmul(
                                sum_ps,
                                lhsT=ow_sbs[b][:, hl, kt : kt + 2, :],
                                rhs=ph,
                                start=(kt == 0),
                                stop=(kt == 14),
                                perf_mode=DR,
                            )
                        extra_mm(2 * kg)
                        extra_mm(2 * kg + 1)
                    rec_t = attp.tile([P, 512], F32, tag="rec", bufs=2)
                    nc.vector.reciprocal_approx_fast(rec_t, sum_ps)
                    ctxn_t = attp.tile([P, 512], FP8, tag="ctxn", bufs=2)
                    nc.vector.tensor_mul(ctxn_t, ctx_ps, rec_t)
                    dma_ctx(b, qc, hl, ctxn_t)

                # ---------- phase 1: QKV(b0), standalone ----------
                with (
                    tc.tile_pool(name="qk0", bufs=1) as qk0p,
                    tc.tile_pool(name="v0", bufs=1) as v0p,
                    tc.tile_pool(name="ow0", bufs=1) as ow0p,
                    tc.tile_pool(name="wq", bufs=1) as wqp,
                    tc.tile_pool(name="hid", bufs=2) as hidp,
                    tc.tile_pool(name="qps", bufs=2, space="PSUM") as qps,
                ):
                    qk_sbs[0] = qk0p.tile([P, 2 * HPC, S], FP8, name="qksb0")
                    v_sbs[0] = v0p.tile([P, HPC, 16, P], FP8, name="vsb0")
                    ow_sbs[0] = ow0p.tile([P, HPC, 16, P], FP8, name="owsb0")
                    # wq on sync (HWDGE, ahead of the hid chunks), first
                    # h-tile pair split out so the first matmuls start early
                    wq_sb = wqp.tile([P, 16, OSH], FP8)
                    nc.sync.dma_start(
                        wq_sb[:, 0:2, :],
                        wqkvT[0 : 2 * P, :].rearrange("(ht p) o -> p ht o", p=P),
                    )
                    nc.sync.dma_start(
                        wq_sb[:, 2:16, :],
                        wqkvT[2 * P :, :].rearrange("(ht p) o -> p ht o", p=P),
                    )

                    def qkv_sc(b, sc, vT_sb):
                        """QKV for one 512-wide s-chunk: 6 o-tiles x 8 h-tile
                        pairs (fp8 DoubleRow); call emit(j) for j in
                        range(48).  V^T o-tiles are PE-transposed to V [k, d]
                        and scaled by exp(alibi[k]); ones' tiles built
                        alongside with exp(alibi[k])/16."""
                        hid_t = hidp.tile([P, 16, 512], FP8, tag="hid")
                        if b == 0 and sc == 0:
                            # split so the first emits only wait half the load
                            for half in range(2):
                                nc.sync.dma_start(
                                    hid_t[:, 8 * half : 8 * half + 8, :],
                                    hidT[0][1024 * half : 1024 * half + 1024, :]
                                    .rearrange("(ht p) q -> p ht q", p=P),
                                )
                        else:
                            nc.sync.dma_start(
                                hid_t,
                                hidT[b * 4 + sc].rearrange("(ht p) q -> p ht q", p=P),
                            )
                        state = {"ps": None}

                        def emit(j):
                            ot, tp = divmod(j, 8)
                            hl, t = divmod(ot, 3)
                            if tp == 0:
                                state["ps"] = qps.tile(
                                    [P, 512], F32, tag="qkvps", bufs=2,
                                    name=f"qps_{b}_{sc}_{ot}",
                                )
                            nc.tensor.matmul(
                                state["ps"],
                                lhsT=wq_sb[:, 2 * tp : 2 * tp + 2, ot * P : (ot + 1) * P],
                                rhs=hid_t[:, 2 * tp : 2 * tp + 2, :],
                                start=(tp == 0),
                                stop=(tp == 7),
                                perf_mode=DR,
                            )
                            if tp == 7:
                                # evacuate on DVE: descale (1/64) + bias in one
                                # dual-scalar op; keeps ScalarE exclusively on Exp
                                dst = (
                                    vT_sb[:, hl, sc * 512 : (sc + 1) * 512]
                                    if t == 2
                                    else qk_sbs[b][:, hl * 2 + t, sc * 512 : (sc + 1) * 512]
                                )
                                nc.vector.tensor_scalar(
                                    out=dst,
                                    in0=state["ps"],
                                    scalar1=1.0 / WSCALE,
                                    scalar2=bq_sb[:, ot : ot + 1],
                                    op0=ALU.mult,
                                    op1=ALU.add,
                                )
                                if t == 2:
                                    # V^T chunk ready: PE-transpose its 4
                                    # k-tiles (psum slots borrowed from the
                                    # qkv pool) and scale rows by exp(alibi)
                                    for kk in range(4):
                                        kt = sc * 4 + kk
                                        acol = (b * HPC + hl) * 16 + kt
                                        vt_ps = qps.tile(
                                            [P, P], BF16, tag="qkvps", bufs=2,
                                            name=f"vt_{b}_{sc}_{hl}_{kk}",
                                        )
                                        nc.tensor.transpose(
                                            vt_ps,
                                            vT_sb[:, hl, kt * P : (kt + 1) * P],
                                            eye_sb,
                                        )
                                        nc.vector.tensor_scalar_mul(
                                            v_sbs[b][:, hl, kt, :],
                                            vt_ps,
                                            ale_sb[:, acol : acol + 1],
                                        )

                        return emit

                    vT0 = vtp.tile([P, HPC, S], BF16, tag="vT", name="vT0")
                    mask_ts = [None] * 4
                    for sc in range(4):
                        emit = qkv_sc(0, sc, vT0)
                        for j in range(48):
                            emit(j)
                        # one mask chunk per s-chunk: spreads the gpsimd DMAs
                        # so chunk 0 lands well before phase 2 needs it
                        mask_ts[sc] = load_mask(sc)
                        if sc == 0:
                            for b in range(B):
                                nc.gpsimd.dma_start(
                                    ow_sbs[b].rearrange("p hl kt c -> p (hl kt c)"),
                                    owT[:, b * 4096 : (b + 1) * 4096],
                                )

                    # dense weights + residual early: plenty of DMA slack
                    # during phase 2, and it removes the phase-3 entry stall
                    wd_sb = dwp.tile([P, 16, DSH], FP8)
                    nc.gpsimd.dma_start(wd_sb, wdT.rearrange("(ht p) o -> p ht o", p=P))
                    wd2_sb = dwp.tile([P, 16, DSH], FP8)
                    nc.gpsimd.dma_start(wd2_sb, wdT2.rearrange("(ht p) o -> p ht o", p=P))
                    rs_sb = dwp.tile([P, 2, B * S], F32)
                    nc.gpsimd.dma_start(
                        rs_sb[:, :, 0:S],
                        residT[:, 0:S].rearrange("(ot p) s -> p ot s", p=P),
                    )

                    # ---------- phase 2: attention(b0) + QKV(b1) ----------
                    # (attn pools span phases 2+3: closing/reopening them at
                    # the phase boundary costs a ~7us all-engine sem barrier)
                    if True:
                        vT1 = vtp.tile([P, HPC, S], BF16, tag="vT", name="vT1")
                        for qc in range(4):
                            for hl in range(HPC):
                                # 48 QKV(b1) DoubleRow matmuls woven into each
                                # block: 6 MMs per k-tile pair.
                                if hl == 0:
                                    emit = qkv_sc(1, qc, vT1)
                                base = 24 * hl

                                def extra(kp, emit=emit, base=base):
                                    for j in range(3):
                                        emit(base + kp * 3 + j)

                                attn_block(0, qc, hl, mask_ts[qc], aps, attp, extra)
                            if qc == 1:
                                all_gather0(0)
                        all_gather0(1)

                # ---------- phase 3: attention(b1) + dense(b0 + b1 early) --
                with (
                    tc.tile_pool(name="dctx", bufs=4) as dctxp,
                    tc.tile_pool(name="dps", bufs=2, space="PSUM") as dps,
                    tc.tile_pool(name="dout", bufs=3) as doutp,
                ):
                    # b1 residual (b0 half already resident); issued on
                    # gpsimd before any phase-3 collective trigger
                    nc.gpsimd.dma_start(
                        rs_sb[:, :, S : 2 * S],
                        residT[:, S : 2 * S].rearrange("(ot p) s -> p ot s", p=P),
                    )

                    def dense_src(sc, g):
                        """(tensor, col offset, row-pair index, lhsT weights)
                        for h-tile-pair group g of output chunk sc."""
                        if sc < 4:
                            return cc_out0[sc // 2], (sc % 2) * 512, g, wd_sb
                        # per-head gathers: g 0-3 = even heads, 4-7 = odd;
                        # wd2 rows are permuted to match
                        return cc_out1[sc - 4][g // 4], 0, g % 4, wd2_sb

                    def dense_sc(sc):
                        """One 512-wide output column chunk: 8 h-tile-pair
                        groups x 2 o-tiles (fp8 DoubleRow); emit(j) for j in
                        range(16)."""
                        state = {}

                        def emit(j):
                            g, ot = divmod(j, 2)
                            src, col_off, gg, wsb = dense_src(sc, g)
                            if ot == 0:
                                state["ctx"] = dctxp.tile(
                                    [P, 2, 512], FP8, tag="dctx", name="dctx_t"
                                )
                                nc.sync.dma_start(
                                    state["ctx"],
                                    src[
                                        gg * 2 * P : (gg + 1) * 2 * P,
                                        col_off : col_off + 512,
                                    ].rearrange("(a p) q -> p a q", p=P),
                                )
                            if g == 0:
                                state[f"ps{ot}"] = dps.tile(
                                    [P, 512], F32, tag="dps", bufs=2,
                                    name=f"dps_{sc}_{ot}",
                                )
                            nc.tensor.matmul(
                                state[f"ps{ot}"],
                                lhsT=wsb[:, 2 * g : 2 * g + 2, ot * P : (ot + 1) * P],
                                rhs=state["ctx"],
                                start=(g == 0),
                                stop=(g == 7),
                                perf_mode=DR,
                            )
                            if j == 15:
                                for o in range(2):
                                    # descale 1/(64*16) on ACT (Copy shares the
                                    # Exp table: no table reload), residual add
                                    # on DVE
                                    t_t = doutp.tile([P, 512], F32, tag="o")
                                    nc.scalar.activation(
                                        t_t, state[f"ps{o}"], AF.Copy,
                                        scale=1.0 / (WSCALE / OWSCALE),
                                    )
                                    o_t = doutp.tile([P, 512], F32, tag="o")
                                    nc.vector.tensor_add(
                                        o_t,
                                        t_t,
                                        rs_sb[:, o, sc * 512 : (sc + 1) * 512],
                                    )
                                    nc.sync.dma_start(
                                        outT[o * P : (o + 1) * P, sc * 512 : (sc + 1) * 512],
                                        o_t,
                                    )

                        return emit

                    if True:
                        # blocks 0..7 = (qc, hl); dense chunks sc0..sc5
                        # woven into blocks 2..7 (2 MMs per k-tile pair),
                        # leaving each gather time to land before use.
                        DENSE_AT = {2: 0, 3: 1, 4: 2, 5: 3, 6: 4, 7: 5}
                        for qc in range(4):
                            for hl in range(HPC):
                                blk = qc * 2 + hl
                                if blk in DENSE_AT:
                                    emit = dense_sc(DENSE_AT[blk])

                                    def extra(kp, emit=emit):
                                        for j in range(2):
                                            emit(kp * 2 + j)
                                else:
                                    def extra(kp):
                                        pass
                                attn_block(1, qc, hl, mask_ts[qc], aps, attp, extra)
                                # gather this head's ctx immediately
                                all_gather1(qc, hl)

                    # ---------- phase 4: dense tail (last b1 columns) ------
                    for sc in range(6, 8):
                        emit = dense_sc(sc)
                        for j in range(16):
                            emit(j)

    nc.compile()
    return nc


def _prep_in_maps(hidden_states, residual, alibi, attention_mask, w_qkv, b_qkv, w_dense, b_dense):
    f32 = np.float32

    def to_fp8(x):
        return np.clip(x, -240.0, 240.0).astype(NPFP8)

    hs = np.asarray(hidden_states, f32).reshape(B * S, H)
    # packed per 512-wide s-chunk: [B*4, H, 512]
    hidT = to_fp8(
        np.ascontiguousarray(hs.T.reshape(H, B * 4, 512).transpose(1, 0, 2))
    )
    mask_keep = ~np.asarray(attention_mask).reshape(S, S)
    # transposed [k, q], packed per 512-wide q-chunk: [4, S, 512].  The mask
    # carries e^-2 so the total exp shift is 5.5 (max score is 10.24; fp8
    # probs must stay under 240).  A uniform scale cancels in ctx/sum.
    mask01T = np.ascontiguousarray(
        mask_keep.T.reshape(S, 4, 512).transpose(1, 0, 2) * np.exp(-2.0)
    ).astype(NPBF16)
    al = np.asarray(alibi, f32).reshape(B, NH, S)
    resid = np.asarray(residual, f32).reshape(B * S, H)
    wq = np.asarray(w_qkv, f32)
    bq = np.asarray(b_qkv, f32)
    wd = np.asarray(w_dense, f32)
    bd = np.asarray(b_dense, f32)

    # h-tile row permutation for the per-head b1 gathers: even h-tiles
    # (heads 0,2,..) first, then odd
    perm = [*range(0, 16, 2), *range(1, 16, 2)]

    in_maps = []
    for r in range(NCORES):
        wshard = wq[r * OSH : (r + 1) * OSH]
        bshard = bq[r * OSH : (r + 1) * OSH]
        alcols = []
        for b in range(B):
            for hl in range(HPC):
                alcols.append(np.exp(al[b, HPC * r + hl]).reshape(16, P).T)
        ale = np.concatenate(alcols, axis=1)
        ow_np = to_fp8(np.repeat(ale[:, :, None] * OWSCALE, P, axis=2).reshape(P, -1))
        wdsh = wd[r * DSH : (r + 1) * DSH].T  # [H, DSH]
        wdsh2 = wdsh.reshape(16, P, DSH)[perm].reshape(H, DSH)
        in_maps.append(
            {
                "hidT": hidT,
                "wqkvT": to_fp8(np.ascontiguousarray(wshard.T) * WSCALE),
                "bqkv": np.ascontiguousarray(bshard.reshape(6, P).T),
                "mask01T": mask01T,
                "alibi_e": np.ascontiguousarray(
                    np.concatenate([ale, ale * OWSCALE], axis=1)
                ),
                "owT": np.ascontiguousarray(ow_np),
                "wdT": to_fp8(np.ascontiguousarray(wdsh) * WSCALE),
                "wdT2": to_fp8(np.ascontiguousarray(wdsh2) * WSCALE),
                "residT": np.ascontiguousarray(resid[:, r * DSH : (r + 1) * DSH].T)
                + bd[r * DSH : (r + 1) * DSH][:, None],
                "eye": np.eye(P, dtype=f32).astype(NPBF16),
            }
        )
    return in_maps


if os.environ.get("BASS_LDW_OPT"):
    _orig_run_command = bass_utils.run_command

    def _run_command_ldwopt(argv, **kwargs):
        argv = [
            "--enable-ldw-opt=true" if a == "--enable-ldw-opt=false" else a
            for a in argv
        ]
        return _orig_run_command(argv, **kwargs)

    bass_utils.run_command = _run_command_ldwopt


_NC_CACHE = {}


def run(inputs: dict, trace: bool = False):
    in_maps = _prep_in_maps(**inputs)
    if "nc" not in _NC_CACHE:
        _NC_CACHE["nc"] = build_nc()
    nc = _NC_CACHE["nc"]
    res = bass_utils.run_bass_kernel_spmd(
        nc, in_maps, core_ids=list(range(NCORES)), trace=trace
    )
    out = np.empty((B * S, H), np.float32)
    for r in range(NCORES):
        out[:, r * DSH : (r + 1) * DSH] = res.results[r]["outT"].T
    return out.reshape(B, S, H), res


def kernel(**inputs) -> np.ndarray:
    out, _ = run(inputs, trace=False)
    return out


# revision 59
# speedup vs baseline: 1.0882x; 1.0882x over previous
# Bloom parallel attention block on 8 trn2 NeuronCores, tensor-parallel over
# heads (2 heads per core).  Feature-major layouts, fp8e4 datapath
# (residual/bias/psum fp32; exp intermediates bf16).
#
# Per core r (heads 2r, 2r+1):
#   QKV matmul in fp8e4 DoubleRow (2 h-tiles per PE op, ~2x bf16/instr):
#     hid fp8, weights host-scaled x64 (fp8 subnormal escape), descaled on
#     the DVE evacuation via dual-scalar tensor_scalar (psum*(1/64)+bias).
#     -> Q^T/K^T [d, s] fp8 and V^T [d, s] bf16 per batch in SBUF.
#     inv_norm (1/sqrt(hd)) is applied in the ACT Exp scale, not the weights.
#   V^T is transposed on the PE to V [s, d] and scaled by exp(alibi[k]) on
#   evacuation (fp8); the softmax-denominator matmul weights are
#   exp(alibi[k])/16 broadcast columns ("ones'", fp8, built on gpsimd).
#   This folds alibi in MULTIPLICATIVELY:
#     exp(s + a) * mask = exp(s) * mask * exp(a)
#   The /16 makes rec = 16/sum so ctx^T*rec lands at fp8-friendly scale.
#   attention (per b, head hl, 512-wide q-chunk qc), scores transposed [k, q]:
#     scores^T = K^T_tile.T @ Q^T       fp8, fp32 psum, per k-tile
#     exp(inv_norm*scores - 3.5)        (ACT, 2 k-tiles/op, bf16 out; the
#                                        mask carries e^-2 more: max score
#                                        is 10.24 and fp8 has no saturation)
#     * mask01^T                        (DVE, [P,4,512] op, -> fp8 probs)
#     ctx^T += V'_pair.T @ prob_pair    (PE fp8 DoubleRow, 8 ops/16 k-tiles)
#     sum   += ones'_pair.T @ prob_pair (PE fp8 DoubleRow, denominator)
#     ctx^T *= 16/sum -> fp8 -> DMA to cc chunk
#   Pipelining: QKV(b1) matmuls are interleaved into attention(b0) k-loops,
#   dense matmuls into attention(b1) k-loops, so the PE never idles.  The
#   mask stays fully resident (shared by both batches); the sync queue
#   carries only wq/hid/ctxn/dctx/out, gpsimd carries collectives + cold
#   loads (HWDGE DMAs occupy their queue for the whole transfer).  ctx is
#   AllGathered in fp8: b0 in 2 column chunks, b1 in 8 per-(qc,head)
#   chunks so the final gather is small and the tail drains fast.
#   dense: fp8 DoubleRow (wd host-scaled x64; ctx fp8 carries x16; wdT2 is
#     row-permuted even/odd to match the per-head b1 gather layout):
#     out^T[o_local, s] = wdT_tile.T @ ctx^T_full; evac = ACT Copy
#     (psum/1024) + DVE add of (residual^T + b_dense).
#     (column-parallel => no all-reduce; host concatenates output slices)
import os
import sys

import numpy as np

if "/opt/trn_rl_repo" not in sys.path:
    sys.path.insert(0, "/opt/trn_rl_repo")

import ml_dtypes

import concourse.bass as bass
import concourse.mybir as mybir
import concourse.tile as tile
from concourse import bacc, bass_utils

B, S, H, NH = 2, 2048, 2048, 16
HD = H // NH            # 128
NCORES = 8
HPC = NH // NCORES      # heads per core = 2
OSH = 3 * H // NCORES   # qkv output rows per core = 768
DSH = H // NCORES       # dense output cols per core = 256
P = 128
F32 = mybir.dt.float32
BF16 = mybir.dt.bfloat16
FP8 = mybir.dt.float8e4
AF = mybir.ActivationFunctionType
ALU = mybir.AluOpType
DR = mybir.MatmulPerfMode.DoubleRow
NPBF16 = ml_dtypes.bfloat16
NPFP8 = ml_dtypes.float8_e4m3

WSCALE = 64.0           # host scale on wq/wd to escape fp8 subnormals
OWSCALE = 1.0 / 16.0    # ones' scale => ctx fp8 carries x16
INV_NORM = 1.0 / np.sqrt(HD)


def build_nc():
    nc = bacc.Bacc(
        "TRN2",
        target_bir_lowering=False,
        debug=False,
        num_devices=NCORES,
    )

    # hidT packed per 512-wide s-chunk (contiguous 1MB per chunk => long DMA
    # descriptors); mask01T packed per q-chunk likewise
    hidT = nc.dram_tensor("hidT", [B * 4, H, 512], FP8, kind="ExternalInput").ap()
    wqkvT = nc.dram_tensor("wqkvT", [H, OSH], FP8, kind="ExternalInput").ap()
    bqkv = nc.dram_tensor("bqkv", [P, 6], F32, kind="ExternalInput").ap()
    mask01T = nc.dram_tensor("mask01T", [4, S, 512], BF16, kind="ExternalInput").ap()
    # cols 0-63: exp(alibi) (V' scale); cols 64-127: exp(alibi)/16 (ones')
    alibi_e = nc.dram_tensor("alibi_e", [P, 4 * HPC * 16], F32, kind="ExternalInput").ap()
    # ones' tiles prebuilt on host (exp(alibi[k])/16 broadcast across 128
    # cols, fp8): building them on-device costs 64 DVE/gpsimd ops
    owT = nc.dram_tensor("owT", [P, B * HPC * 16 * P], FP8, kind="ExternalInput").ap()
    wdT = nc.dram_tensor("wdT", [H, DSH], FP8, kind="ExternalInput").ap()
    # h-tile rows permuted [0,2,..,14,1,3,..,15] for the per-head b1 gathers
    wdT2 = nc.dram_tensor("wdT2", [H, DSH], FP8, kind="ExternalInput").ap()
    residT = nc.dram_tensor("residT", [DSH, B * S], F32, kind="ExternalInput").ap()
    eye = nc.dram_tensor("eye", [P, P], BF16, kind="ExternalInput").ap()
    outT = nc.dram_tensor("outT", [DSH, B * S], F32, kind="ExternalOutput").ap()

    with tile.TileContext(nc) as tc:
        ccg = [list(range(NCORES))]
        with (
            tc.tile_pool(name="const", bufs=1) as constp,
            tc.tile_pool(name="dram", bufs=1, space="DRAM") as dramp,
        ):
            bq_sb = constp.tile([P, 6], F32)
            nc.gpsimd.dma_start(bq_sb, bqkv)
            # shared bias column for the exps: passing a float bias makes
            # bass materialize a const AP per activation call (~25us of DVE
            # setup for 128 exps).  The -3.5 shift keeps exp outputs under
            # fp8e4's 240 max normal (measured score max ~7.8 sigma, and
            # the fp8 cast does NOT saturate: overflow becomes inf); the
            # shift cancels in the ctx/sum ratio.
            shift_col = constp.tile([P, 1], F32)
            nc.vector.memset(shift_col, -3.5)
            ale_sb = constp.tile([P, 4 * HPC * 16], F32)
            nc.gpsimd.dma_start(ale_sb, alibi_e)
            eye_sb = constp.tile([P, P], BF16)
            nc.gpsimd.dma_start(eye_sb, eye)

            # ctx gather chunks (fp8): b0 in 4 per-qc chunks [2 heads x 512]
            # (gathered as soon as each q-chunk completes in phase 2),
            # b1 in 8 per-(qc, head) chunks [1 head x 512] so the last gather
            # is tiny and the pipeline drains quickly at the tail.
            cc_in0 = [
                dramp.tile([HPC * HD, 512], FP8, name=f"cc_in0{i}")
                for i in range(4)
            ]
            cc_out0 = [
                dramp.tile([NCORES * HPC * HD, 512], FP8, addr_space="Shared",
                           name=f"cc_out0{i}")
                for i in range(4)
            ]
            cc_in1 = [
                [dramp.tile([HD, 512], FP8, name=f"cc_in1{qc}{hl}") for hl in range(HPC)]
                for qc in range(4)
            ]
            cc_out1 = [
                [
                    dramp.tile([NCORES * HD, 512], FP8, addr_space="Shared",
                               name=f"cc_out1{qc}{hl}")
                    for hl in range(HPC)
                ]
                for qc in range(4)
            ]

            def dma_ctx(b, qc, hl, ctxn_t):
                if b == 0:
                    nc.sync.dma_start(cc_in0[qc][hl * P : (hl + 1) * P, :], ctxn_t)
                else:
                    nc.sync.dma_start(cc_in1[qc][hl], ctxn_t)

            def all_gather0(chunk):
                nc.gpsimd.collective_compute(
                    "AllGather", ALU.bypass, replica_groups=ccg,
                    ins=[cc_in0[chunk].opt()], outs=[cc_out0[chunk].opt()],
                )

            def all_gather1(qc, hl):
                nc.gpsimd.collective_compute(
                    "AllGather", ALU.bypass, replica_groups=ccg,
                    ins=[cc_in1[qc][hl].opt()], outs=[cc_out1[qc][hl].opt()],
                )

            with (
                tc.tile_pool(name="mask", bufs=4) as maskp,
                tc.tile_pool(name="qk1", bufs=1) as qk1p,
                tc.tile_pool(name="vt", bufs=1) as vtp,
                tc.tile_pool(name="v1", bufs=1) as v1p,
                tc.tile_pool(name="ow1", bufs=1) as ow1p,
                tc.tile_pool(name="dw", bufs=1) as dwp,
                tc.tile_pool(name="att", bufs=1) as attp,
                tc.tile_pool(name="aps", bufs=1, space="PSUM") as aps,
            ):
                qk_sbs = [None, qk1p.tile([P, 2 * HPC, S], FP8, name="qksb1")]
                v_sbs = [None, v1p.tile([P, HPC, 16, P], FP8, name="vsb1")]
                ow_sbs = [None, ow1p.tile([P, HPC, 16, P], FP8, name="owsb1")]

                def load_mask(qc):
                    """All 4 mask q-chunks stay resident (the mask is shared
                    by both batches, so phases 2 and 3 reuse the same tiles
                    and NO mask DMA competes with compute).  Loaded on
                    gpsimd at phase-1 end, before any collective trigger
                    can block that queue."""
                    m = maskp.tile([P, 16, 512], BF16, tag="mask")
                    nc.gpsimd.dma_start(
                        m,
                        mask01T[qc].rearrange("(kt p) q -> p kt q", p=P),
                    )
                    return m

                def attn_block(b, qc, hl, mask_sb, aps, attp, extra_mm):
                    """Attention for (b, head hl, q-chunk qc).  k-tiles are
                    processed in groups of 4 (2 score-psum pairs, 1 wide mask
                    mul); extra_mm(kp) for kp in 0..7 emits independent
                    matmuls to keep the PE busy while ACT/DVE run."""
                    qk = qk_sbs[b]
                    ctx_ps = aps.tile([P, 512], F32, tag="ctx", bufs=1)
                    sum_ps = aps.tile([P, 512], F32, tag="sum", bufs=1)
                    for kg in range(4):
                        kt0 = 4 * kg
                        exp_t = attp.tile([P, 4, 512], BF16, tag="exp", bufs=2)
                        for u2 in range(2):
                            s_ps = aps.tile([P, 1024], F32, tag="sco", bufs=2)
                            for u in range(2):
                                kt = kt0 + 2 * u2 + u
                                nc.tensor.matmul(
                                    s_ps[:, u * 512 : (u + 1) * 512],
                                    lhsT=qk[:, hl * 2 + 1, kt * P : (kt + 1) * P],
                                    rhs=qk[:, hl * 2, qc * 512 : (qc + 1) * 512],
                                    start=True,
                                    stop=True,
                                )
                            nc.scalar.activation(
                                exp_t[:, 2 * u2 : 2 * u2 + 2, :],
                                s_ps.rearrange("p (u q) -> p u q", u=2),
                                AF.Exp,
                                bias=shift_col[:, 0:1],
                                scale=float(INV_NORM),
                            )
                        prob_t = attp.tile([P, 4, 512], FP8, tag="prob", bufs=2)
                        nc.vector.tensor_mul(
                            prob_t,
                            exp_t,
                            mask_sb[:, kt0 : kt0 + 4, :],
                        )
                        for u2 in range(2):
                            kt = kt0 + 2 * u2
                            ph = prob_t[:, 2 * u2 : 2 * u2 + 2, :]
                            nc.tensor.matmul(
                                ctx_ps,
                                lhsT=v_sbs[b][:, hl, kt : kt + 2, :],
                                rhs=ph,
                                start=(kt == 0),
                                stop=(kt == 14),
                                perf_mode=DR,
                            )
                            nc.tensor.matmul(
                                sum_ps,
                                lhsT=ow_sbs[b][:, hl, kt : kt + 2, :],
                                rhs=ph,
                                start=(kt == 0),
                                stop=(kt == 14),
                                perf_mode=DR,
                            )
                        extra_mm(2 * kg)
                        extra_mm(2 * kg + 1)
                    rec_t = attp.tile([P, 512], F32, tag="rec", bufs=2)
                    nc.vector.reciprocal_approx_fast(rec_t, sum_ps)
                    ctxn_t = attp.tile([P, 512], FP8, tag="ctxn", bufs=2)
                    nc.vector.tensor_mul(ctxn_t, ctx_ps, rec_t)
                    dma_ctx(b, qc, hl, ctxn_t)

                # ---------- phase 1: QKV(b0), standalone ----------
                with (
                    tc.tile_pool(name="qk0", bufs=1) as qk0p,
                    tc.tile_pool(name="v0", bufs=1) as v0p,
                    tc.tile_pool(name="ow0", bufs=1) as ow0p,
                    tc.tile_pool(name="wq", bufs=1) as wqp,
                    tc.tile_pool(name="hid", bufs=2) as hidp,
                    tc.tile_pool(name="qps", bufs=2, space="PSUM") as qps,
                ):
                    qk_sbs[0] = qk0p.tile([P, 2 * HPC, S], FP8, name="qksb0")
                    v_sbs[0] = v0p.tile([P, HPC, 16, P], FP8, name="vsb0")
                    ow_sbs[0] = ow0p.tile([P, HPC, 16, P], FP8, name="owsb0")
                    # wq on sync (HWDGE, ahead of the hid chunks), first
                    # h-tile pair split out so the first matmuls start early
                    wq_sb = wqp.tile([P, 16, OSH], FP8)
                    nc.sync.dma_start(
                        wq_sb[:, 0:2, :],
                        wqkvT[0 : 2 * P, :].rearrange("(ht p) o -> p ht o", p=P),
                    )
                    nc.sync.dma_start(
                        wq_sb[:, 2:16, :],
                        wqkvT[2 * P :, :].rearrange("(ht p) o -> p ht o", p=P),
                    )

                    def qkv_sc(b, sc, vT_sb):
                        """QKV for one 512-wide s-chunk: 6 o-tiles x 8 h-tile
                        pairs (fp8 DoubleRow); call emit(j) for j in
                        range(48).  V^T o-tiles are PE-transposed to V [k, d]
                        and scaled by exp(alibi[k]); ones' tiles built
                        alongside with exp(alibi[k])/16."""
                        hid_t = hidp.tile([P, 16, 512], FP8, tag="hid")
                        if b == 0 and sc == 0:
                            # split so the first emits only wait half the load
                            for half in range(2):
                                nc.sync.dma_start(
                                    hid_t[:, 8 * half : 8 * half + 8, :],
                                    hidT[0][1024 * half : 1024 * half + 1024, :]
                                    .rearrange("(ht p) q -> p ht q", p=P),
                                )
                        else:
                            nc.sync.dma_start(
                                hid_t,
                                hidT[b * 4 + sc].rearrange("(ht p) q -> p ht q", p=P),
                            )
                        state = {"ps": None}

                        def emit(j):
                            ot, tp = divmod(j, 8)
                            hl, t = divmod(ot, 3)
                            if tp == 0:
                                state["ps"] = qps.tile(
                                    [P, 512], F32, tag="qkvps", bufs=2,
                                    name=f"qps_{b}_{sc}_{ot}",
                                )
                            nc.tensor.matmul(
                                state["ps"],
                                lhsT=wq_sb[:, 2 * tp : 2 * tp + 2, ot * P : (ot + 1) * P],
                                rhs=hid_t[:, 2 * tp : 2 * tp + 2, :],
                                start=(tp == 0),
                                stop=(tp == 7),
                                perf_mode=DR,
                            )
                            if tp == 7:
                                # evacuate on DVE: descale (1/64) + bias in one
                                # dual-scalar op; keeps ScalarE exclusively on Exp
                                dst = (
                                    vT_sb[:, hl, sc * 512 : (sc + 1) * 512]
                                    if t == 2
                                    else qk_sbs[b][:, hl * 2 + t, sc * 512 : (sc + 1) * 512]
                                )
                                nc.vector.tensor_scalar(
                                    out=dst,
                                    in0=state["ps"],
                                    scalar1=1.0 / WSCALE,
                                    scalar2=bq_sb[:, ot : ot + 1],
                                    op0=ALU.mult,
                                    op1=ALU.add,
                                )
                                if t == 2:
                                    # V^T chunk ready: PE-transpose its 4
                                    # k-tiles (psum slots borrowed from the
                                    # qkv pool) and scale rows by exp(alibi)
                                    for kk in range(4):
                                        kt = sc * 4 + kk
                                        acol = (b * HPC + hl) * 16 + kt
                                        vt_ps = qps.tile(
                                            [P, P], BF16, tag="qkvps", bufs=2,
                                            name=f"vt_{b}_{sc}_{hl}_{kk}",
                                        )
                                        nc.tensor.transpose(
                                            vt_ps,
                                            vT_sb[:, hl, kt * P : (kt + 1) * P],
                                            eye_sb,
                                        )
                                        nc.vector.tensor_scalar_mul(
                                            v_sbs[b][:, hl, kt, :],
                                            vt_ps,
                                            ale_sb[:, acol : acol + 1],
                                        )

                        return emit

                    vT0 = vtp.tile([P, HPC, S], BF16, tag="vT", name="vT0")
                    mask_ts = [None] * 4
                    for sc in range(4):
                        emit = qkv_sc(0, sc, vT0)
                        for j in range(48):
                            emit(j)
                        # one mask chunk per s-chunk: spreads the gpsimd DMAs
                        # so chunk 0 lands well before phase 2 needs it
                        mask_ts[sc] = load_mask(sc)
                        if sc == 0:
                            for b in range(B):
                                nc.gpsimd.dma_start(
                                    ow_sbs[b].rearrange("p hl kt c -> p (hl kt c)"),
                                    owT[:, b * 4096 : (b + 1) * 4096],
                                )

                    # dense weights + residual early: plenty of DMA slack
                    # during phase 2, and it removes the phase-3 entry stall
                    wd_sb = dwp.tile([P, 16, DSH], FP8)
                    nc.gpsimd.dma_start(wd_sb, wdT.rearrange("(ht p) o -> p ht o", p=P))
                    wd2_sb = dwp.tile([P, 16, DSH], FP8)
                    nc.gpsimd.dma_start(wd2_sb, wdT2.rearrange("(ht p) o -> p ht o", p=P))
                    rs_sb = dwp.tile([P, 2, B * S], F32)
                    nc.gpsimd.dma_start(
                        rs_sb[:, :, 0:S],
                        residT[:, 0:S].rearrange("(ot p) s -> p ot s", p=P),
                    )

                    # ---------- phase 2: attention(b0) + QKV(b1) ----------
                    # (attn pools span phases 2+3: closing/reopening them at
                    # the phase boundary costs a ~7us all-engine sem barrier)
                    if True:
                        vT1 = vtp.tile([P, HPC, S], BF16, tag="vT", name="vT1")
                        for qc in range(4):
                            for hl in range(HPC):
                                # 48 QKV(b1) DoubleRow matmuls woven into each
                                # block: 6 MMs per k-tile pair.
                                if hl == 0:
                                    emit = qkv_sc(1, qc, vT1)
                                base = 24 * hl

                                def extra(kp, emit=emit, base=base):
                                    for j in range(3):
                                        emit(base + kp * 3 + j)

                                attn_block(0, qc, hl, mask_ts[qc], aps, attp, extra)
                            # gather per q-chunk: early fire = slack against
                            # inter-core skew inflating collective latency
                            all_gather0(qc)

                # ---------- phase 3: attention(b1) + dense(b0 + b1 early) --
                with (
                    tc.tile_pool(name="dctx", bufs=4) as dctxp,
                    tc.tile_pool(name="dps", bufs=2, space="PSUM") as dps,
                    tc.tile_pool(name="dout", bufs=3) as doutp,
                ):
                    # b1 residual (b0 half already resident); issued on
                    # gpsimd before any phase-3 collective trigger
                    nc.gpsimd.dma_start(
                        rs_sb[:, :, S : 2 * S],
                        residT[:, S : 2 * S].rearrange("(ot p) s -> p ot s", p=P),
                    )

                    def dense_src(sc, g):
                        """(tensor, col offset, row-pair index, lhsT weights)
                        for h-tile-pair group g of output chunk sc."""
                        if sc < 4:
                            return cc_out0[sc], 0, g, wd_sb
                        # per-head gathers: g 0-3 = even heads, 4-7 = odd;
                        # wd2 rows are permuted to match
                        return cc_out1[sc - 4][g // 4], 0, g % 4, wd2_sb

                    def dense_sc(sc):
                        """One 512-wide output column chunk: 8 h-tile-pair
                        groups x 2 o-tiles (fp8 DoubleRow); emit(j) for j in
                        range(16)."""
                        state = {}

                        def emit(j):
                            g, ot = divmod(j, 2)
                            src, col_off, gg, wsb = dense_src(sc, g)
                            if ot == 0:
                                state["ctx"] = dctxp.tile(
                                    [P, 2, 512], FP8, tag="dctx", name="dctx_t"
                                )
                                nc.sync.dma_start(
                                    state["ctx"],
                                    src[
                                        gg * 2 * P : (gg + 1) * 2 * P,
                                        col_off : col_off + 512,
                                    ].rearrange("(a p) q -> p a q", p=P),
                                )
                            if g == 0:
                                state[f"ps{ot}"] = dps.tile(
                                    [P, 512], F32, tag="dps", bufs=2,
                                    name=f"dps_{sc}_{ot}",
                                )
                            nc.tensor.matmul(
                                state[f"ps{ot}"],
                                lhsT=wsb[:, 2 * g : 2 * g + 2, ot * P : (ot + 1) * P],
                                rhs=state["ctx"],
                                start=(g == 0),
                                stop=(g == 7),
                                perf_mode=DR,
                            )
                            if j == 15:
                                for o in range(2):
                                    # descale 1/(64*16) on ACT (Copy shares the
                                    # Exp table: no table reload), residual add
                                    # on DVE
                                    t_t = doutp.tile([P, 512], F32, tag="o")
                                    nc.scalar.activation(
                                        t_t, state[f"ps{o}"], AF.Copy,
                                        scale=1.0 / (WSCALE / OWSCALE),
                                    )
                                    o_t = doutp.tile([P, 512], F32, tag="o")
                                    nc.vector.tensor_add(
                                        o_t,
                                        t_t,
                                        rs_sb[:, o, sc * 512 : (sc + 1) * 512],
                                    )
                                    nc.sync.dma_start(
                                        outT[o * P : (o + 1) * P, sc * 512 : (sc + 1) * 512],
                                        o_t,
                                    )

                        return emit

                    if True:
                        # blocks 0..7 = (qc, hl); dense chunks sc0..sc5
                        # woven into blocks 2..7 (2 MMs per k-tile pair),
                        # leaving each gather time to land before use.
                        DENSE_AT = {2: 0, 3: 1, 4: 2, 5: 3, 6: 4, 7: 5}
                        for qc in range(4):
                            for hl in range(HPC):
                                blk = qc * 2 + hl
                                if blk in DENSE_AT:
                                    emit = dense_sc(DENSE_AT[blk])

                                    def extra(kp, emit=emit):
                                        for j in range(2):
                                            emit(kp * 2 + j)
                                else:
                                    def extra(kp):
                                        pass
                                attn_block(1, qc, hl, mask_ts[qc], aps, attp, extra)
                                # gather this head's ctx immediately
                                all_gather1(qc, hl)

                    # ---------- phase 4: dense tail (last b1 columns) ------
                    for sc in range(6, 8):
                        emit = dense_sc(sc)
                        for j in range(16):
                            emit(j)

    nc.compile()
    return nc


def _prep_in_maps(hidden_states, residual, alibi, attention_mask, w_qkv, b_qkv, w_dense, b_dense):
    f32 = np.float32

    def to_fp8(x):
        return np.clip(x, -240.0, 240.0).astype(NPFP8)

    hs = np.asarray(hidden_states, f32).reshape(B * S, H)
    # packed per 512-wide s-chunk: [B*4, H, 512]
    hidT = to_fp8(
        np.ascontiguousarray(hs.T.reshape(H, B * 4, 512).transpose(1, 0, 2))
    )
    mask_keep = ~np.asarray(attention_mask).reshape(S, S)
    # transposed [k, q], packed per 512-wide q-chunk: [4, S, 512].  The mask
    # carries e^-2 so the total exp shift is 5.5 (max score is 10.24; fp8
    # probs must stay under 240).  A uniform scale cancels in ctx/sum.
    mask01T = np.ascontiguousarray(
        mask_keep.T.reshape(S, 4, 512).transpose(1, 0, 2) * np.exp(-2.0)
    ).astype(NPBF16)
    al = np.asarray(alibi, f32).reshape(B, NH, S)
    resid = np.asarray(residual, f32).reshape(B * S, H)
    wq = np.asarray(w_qkv, f32)
    bq = np.asarray(b_qkv, f32)
    wd = np.asarray(w_dense, f32)
    bd = np.asarray(b_dense, f32)

    # h-tile row permutation for the per-head b1 gathers: even h-tiles
    # (heads 0,2,..) first, then odd
    perm = [*range(0, 16, 2), *range(1, 16, 2)]

    in_maps = []
    for r in range(NCORES):
        wshard = wq[r * OSH : (r + 1) * OSH]
        bshard = bq[r * OSH : (r + 1) * OSH]
        alcols = []
        for b in range(B):
            for hl in range(HPC):
                alcols.append(np.exp(al[b, HPC * r + hl]).reshape(16, P).T)
        ale = np.concatenate(alcols, axis=1)
        ow_np = to_fp8(np.repeat(ale[:, :, None] * OWSCALE, P, axis=2).reshape(P, -1))
        wdsh = wd[r * DSH : (r + 1) * DSH].T  # [H, DSH]
        wdsh2 = wdsh.reshape(16, P, DSH)[perm].reshape(H, DSH)
        in_maps.append(
            {
                "hidT": hidT,
                "wqkvT": to_fp8(np.ascontiguousarray(wshard.T) * WSCALE),
                "bqkv": np.ascontiguousarray(bshard.reshape(6, P).T),
                "mask01T": mask01T,
                "alibi_e": np.ascontiguousarray(
                    np.concatenate([ale, ale * OWSCALE], axis=1)
                ),
                "owT": np.ascontiguousarray(ow_np),
                "wdT": to_fp8(np.ascontiguousarray(wdsh) * WSCALE),
                "wdT2": to_fp8(np.ascontiguousarray(wdsh2) * WSCALE),
                "residT": np.ascontiguousarray(resid[:, r * DSH : (r + 1) * DSH].T)
                + bd[r * DSH : (r + 1) * DSH][:, None],
                "eye": np.eye(P, dtype=f32).astype(NPBF16),
            }
        )
    return in_maps


if os.environ.get("BASS_LDW_OPT"):
    _orig_run_command = bass_utils.run_command

    def _run_command_ldwopt(argv, **kwargs):
        argv = [
            "--enable-ldw-opt=true" if a == "--enable-ldw-opt=false" else a
            for a in argv
        ]
        return _orig_run_command(argv, **kwargs)

    bass_utils.run_command = _run_command_ldwopt


_NC_CACHE = {}


def run(inputs: dict, trace: bool = False):
    in_maps = _prep_in_maps(**inputs)
    if "nc" not in _NC_CACHE:
        _NC_CACHE["nc"] = build_nc()
    nc = _NC_CACHE["nc"]
    res = bass_utils.run_bass_kernel_spmd(
        nc, in_maps, core_ids=list(range(NCORES)), trace=trace
    )
    out = np.empty((B * S, H), np.float32)
    for r in range(NCORES):
        out[:, r * DSH : (r + 1) * DSH] = res.results[r]["outT"].T
    return out.reshape(B, S, H), res


def kernel(**inputs) -> np.ndarray:
    out, _ = run(inputs, trace=False)
    return out


# revision 60
# speedup vs baseline: 1.1610x; 1.0669x over previous
# Bloom parallel attention block on 8 trn2 NeuronCores, tensor-parallel over
# heads (2 heads per core).  Feature-major layouts, fp8e4 datapath
# (residual/bias/psum fp32; exp intermediates bf16).
#
# Per core r (heads 2r, 2r+1):
#   QKV matmul in fp8e4 DoubleRow (2 h-tiles per PE op, ~2x bf16/instr):
#     hid fp8, weights host-scaled x64 (fp8 subnormal escape), descaled on
#     the DVE evacuation via dual-scalar tensor_scalar (psum*(1/64)+bias).
#     -> Q^T/K^T [d, s] fp8 and V^T [d, s] bf16 per batch in SBUF.
#     inv_norm (1/sqrt(hd)) is applied in the ACT Exp scale, not the weights.
#   V^T is transposed on the PE to V [s, d] and scaled by exp(alibi[k]) on
#   evacuation (fp8); the softmax-denominator matmul weights are
#   exp(alibi[k])/16 broadcast columns ("ones'", fp8, built on gpsimd).
#   This folds alibi in MULTIPLICATIVELY:
#     exp(s + a) * mask = exp(s) * mask * exp(a)
#   The /16 makes rec = 16/sum so ctx^T*rec lands at fp8-friendly scale.
#   attention (per b, head hl, 512-wide q-chunk qc), scores transposed [k, q]:
#     scores^T = K^T_tile.T @ Q^T       fp8, fp32 psum, per k-tile
#     exp(inv_norm*scores - 3.5)        (ACT, 2 k-tiles/op, bf16 out; the
#                                        mask carries e^-2 more: max score
#                                        is 10.24 and fp8 has no saturation)
#     * mask01^T                        (DVE, [P,4,512] op, -> fp8 probs)
#     ctx^T += V'_pair.T @ prob_pair    (PE fp8 DoubleRow, 8 ops/16 k-tiles)
#     sum   += ones'_pair.T @ prob_pair (PE fp8 DoubleRow, denominator)
#     ctx^T *= 16/sum -> fp8 -> DMA to cc chunk
#   Pipelining: QKV(b1) matmuls are interleaved into attention(b0) k-loops,
#   dense matmuls into attention(b1) k-loops, so the PE never idles.  The
#   mask stays fully resident (shared by both batches); the sync queue
#   carries only wq/hid/ctxn/dctx/out, gpsimd carries collectives + cold
#   loads (HWDGE DMAs occupy their queue for the whole transfer).  ctx is
#   AllGathered in fp8: b0 in 2 column chunks, b1 in 8 per-(qc,head)
#   chunks so the final gather is small and the tail drains fast.
#   dense: fp8 DoubleRow (wd host-scaled x64; ctx fp8 carries x16; wdT2 is
#     row-permuted even/odd to match the per-head b1 gather layout):
#     out^T[o_local, s] = wdT_tile.T @ ctx^T_full; evac = ACT Copy
#     (psum/1024) + DVE add of (residual^T + b_dense).
#     (column-parallel => no all-reduce; host concatenates output slices)
import os
import sys

import numpy as np

if "/opt/trn_rl_repo" not in sys.path:
    sys.path.insert(0, "/opt/trn_rl_repo")

import ml_dtypes

import concourse.bass as bass
import concourse.mybir as mybir
import concourse.tile as tile
from concourse import bacc, bass_utils

B, S, H, NH = 2, 2048, 2048, 16
HD = H // NH            # 128
NCORES = 8
HPC = NH // NCORES      # heads per core = 2
OSH = 3 * H // NCORES   # qkv output rows per core = 768
DSH = H // NCORES       # dense output cols per core = 256
P = 128
F32 = mybir.dt.float32
BF16 = mybir.dt.bfloat16
FP8 = mybir.dt.float8e4
AF = mybir.ActivationFunctionType
ALU = mybir.AluOpType
DR = mybir.MatmulPerfMode.DoubleRow
NPBF16 = ml_dtypes.bfloat16
NPFP8 = ml_dtypes.float8_e4m3

WSCALE = 64.0           # host scale on wq/wd to escape fp8 subnormals
OWSCALE = 1.0 / 16.0    # ones' scale => ctx fp8 carries x16
INV_NORM = 1.0 / np.sqrt(HD)


def build_nc():
    nc = bacc.Bacc(
        "TRN2",
        target_bir_lowering=False,
        debug=False,
        num_devices=NCORES,
    )

    # hidT packed per 512-wide s-chunk (contiguous 1MB per chunk => long DMA
    # descriptors); mask01T packed per q-chunk likewise
    hidT = nc.dram_tensor("hidT", [B * 4, H, 512], FP8, kind="ExternalInput").ap()
    wqkvT = nc.dram_tensor("wqkvT", [H, OSH], FP8, kind="ExternalInput").ap()
    bqkv = nc.dram_tensor("bqkv", [P, 6], F32, kind="ExternalInput").ap()
    mask01T = nc.dram_tensor("mask01T", [4, S, 512], BF16, kind="ExternalInput").ap()
    # cols 0-63: exp(alibi) (V' scale); cols 64-127: exp(alibi)/16 (ones')
    alibi_e = nc.dram_tensor("alibi_e", [P, 4 * HPC * 16], F32, kind="ExternalInput").ap()
    wdT = nc.dram_tensor("wdT", [H, DSH], FP8, kind="ExternalInput").ap()
    # h-tile rows permuted [0,2,..,14,1,3,..,15] for the per-head b1 gathers
    wdT2 = nc.dram_tensor("wdT2", [H, DSH], FP8, kind="ExternalInput").ap()
    residT = nc.dram_tensor("residT", [DSH, B * S], F32, kind="ExternalInput").ap()
    ones = nc.dram_tensor("ones", [P, P], BF16, kind="ExternalInput").ap()
    eye = nc.dram_tensor("eye", [P, P], BF16, kind="ExternalInput").ap()
    outT = nc.dram_tensor("outT", [DSH, B * S], F32, kind="ExternalOutput").ap()

    with tile.TileContext(nc) as tc:
        ccg = [list(range(NCORES))]
        with (
            tc.tile_pool(name="const", bufs=1) as constp,
            tc.tile_pool(name="dram", bufs=1, space="DRAM") as dramp,
        ):
            bq_sb = constp.tile([P, 6], F32)
            nc.gpsimd.dma_start(bq_sb, bqkv)
            # shared bias column for the exps: passing a float bias makes
            # bass materialize a const AP per activation call (~25us of DVE
            # setup for 128 exps).  The -3.5 shift keeps exp outputs under
            # fp8e4's 240 max normal (measured score max ~7.8 sigma, and
            # the fp8 cast does NOT saturate: overflow becomes inf); the
            # shift cancels in the ctx/sum ratio.
            shift_col = constp.tile([P, 1], F32)
            nc.vector.memset(shift_col, -3.5)
            ale_sb = constp.tile([P, 4 * HPC * 16], F32)
            nc.gpsimd.dma_start(ale_sb, alibi_e)
            ones_sb = constp.tile(
                [P, P], BF16,
                name="ones_sb_ldw" if os.environ.get("BASS_LDW_OPT") else "ones_sb",
            )
            nc.gpsimd.dma_start(ones_sb, ones)
            eye_sb = constp.tile([P, P], BF16)
            nc.gpsimd.dma_start(eye_sb, eye)

            # ctx gather chunks (fp8): b0 in 4 per-qc chunks [2 heads x 512]
            # (gathered as soon as each q-chunk completes in phase 2),
            # b1 in 8 per-(qc, head) chunks [1 head x 512] so the last gather
            # is tiny and the pipeline drains quickly at the tail.
            cc_in0 = [
                dramp.tile([HPC * HD, S // 2], FP8, name=f"cc_in0{i}")
                for i in range(2)
            ]
            cc_out0 = [
                dramp.tile([NCORES * HPC * HD, S // 2], FP8, addr_space="Shared",
                           name=f"cc_out0{i}")
                for i in range(2)
            ]
            cc_in1 = [
                [dramp.tile([HD, 512], FP8, name=f"cc_in1{qc}{hl}") for hl in range(HPC)]
                for qc in range(4)
            ]
            cc_out1 = [
                [
                    dramp.tile([NCORES * HD, 512], FP8, addr_space="Shared",
                               name=f"cc_out1{qc}{hl}")
                    for hl in range(HPC)
                ]
                for qc in range(4)
            ]

            def dma_ctx(b, qc, hl, ctxn_t):
                if b == 0:
                    chunk, qq = divmod(qc, 2)
                    nc.sync.dma_start(
                        cc_in0[chunk][hl * P : (hl + 1) * P, qq * 512 : (qq + 1) * 512],
                        ctxn_t,
                    )
                else:
                    nc.sync.dma_start(cc_in1[qc][hl], ctxn_t)

            def all_gather0(chunk):
                nc.gpsimd.collective_compute(
                    "AllGather", ALU.bypass, replica_groups=ccg,
                    ins=[cc_in0[chunk].opt()], outs=[cc_out0[chunk].opt()],
                )

            def all_gather1(qc, hl):
                nc.gpsimd.collective_compute(
                    "AllGather", ALU.bypass, replica_groups=ccg,
                    ins=[cc_in1[qc][hl].opt()], outs=[cc_out1[qc][hl].opt()],
                )

            with (
                tc.tile_pool(name="mask", bufs=4) as maskp,
                tc.tile_pool(name="qk1", bufs=1) as qk1p,
                tc.tile_pool(name="vt", bufs=1) as vtp,
                tc.tile_pool(name="v1", bufs=1) as v1p,
                tc.tile_pool(name="ow1", bufs=1) as ow1p,
                tc.tile_pool(name="dw", bufs=1) as dwp,
            ):
                qk_sbs = [None, qk1p.tile([P, 2 * HPC, S], FP8, name="qksb1")]
                v_sbs = [None, v1p.tile([P, HPC, 16, P], FP8, name="vsb1")]
                ow_sbs = [None, ow1p.tile([P, HPC, 16, P], FP8, name="owsb1")]

                def load_mask(qc):
                    """All 4 mask q-chunks stay resident (the mask is shared
                    by both batches, so phases 2 and 3 reuse the same tiles
                    and NO mask DMA competes with compute).  Loaded on
                    gpsimd at phase-1 end, before any collective trigger
                    can block that queue."""
                    m = maskp.tile([P, 16, 512], BF16, tag="mask")
                    nc.gpsimd.dma_start(
                        m,
                        mask01T[qc].rearrange("(kt p) q -> p kt q", p=P),
                    )
                    return m

                def attn_block(b, qc, hl, mask_sb, aps, attp, extra_mm):
                    """Attention for (b, head hl, q-chunk qc).  k-tiles are
                    processed in groups of 4 (2 score-psum pairs, 1 wide mask
                    mul); extra_mm(kp) for kp in 0..7 emits independent
                    matmuls to keep the PE busy while ACT/DVE run."""
                    qk = qk_sbs[b]
                    ctx_ps = aps.tile([P, 512], F32, tag="ctx", bufs=1)
                    sum_ps = aps.tile([P, 512], F32, tag="sum", bufs=1)
                    for kg in range(4):
                        kt0 = 4 * kg
                        exp_t = attp.tile([P, 4, 512], BF16, tag="exp", bufs=2)
                        for u2 in range(2):
                            s_ps = aps.tile([P, 1024], F32, tag="sco", bufs=2)
                            for u in range(2):
                                kt = kt0 + 2 * u2 + u
                                nc.tensor.matmul(
                                    s_ps[:, u * 512 : (u + 1) * 512],
                                    lhsT=qk[:, hl * 2 + 1, kt * P : (kt + 1) * P],
                                    rhs=qk[:, hl * 2, qc * 512 : (qc + 1) * 512],
                                    start=True,
                                    stop=True,
                                )
                            nc.scalar.activation(
                                exp_t[:, 2 * u2 : 2 * u2 + 2, :],
                                s_ps.rearrange("p (u q) -> p u q", u=2),
                                AF.Exp,
                                bias=shift_col[:, 0:1],
                                scale=float(INV_NORM),
                            )
                        prob_t = attp.tile([P, 4, 512], FP8, tag="prob", bufs=2)
                        nc.vector.tensor_mul(
                            prob_t,
                            exp_t,
                            mask_sb[:, kt0 : kt0 + 4, :],
                        )
                        for u2 in range(2):
                            kt = kt0 + 2 * u2
                            ph = prob_t[:, 2 * u2 : 2 * u2 + 2, :]
                            nc.tensor.matmul(
                                ctx_ps,
                                lhsT=v_sbs[b][:, hl, kt : kt + 2, :],
                                rhs=ph,
                                start=(kt == 0),
                                stop=(kt == 14),
                                perf_mode=DR,
                            )
                            nc.tensor.matmul(
                                sum_ps,
                                lhsT=ow_sbs[b][:, hl, kt : kt + 2, :],
                                rhs=ph,
                                start=(kt == 0),
                                stop=(kt == 14),
                                perf_mode=DR,
                            )
                        extra_mm(2 * kg)
                        extra_mm(2 * kg + 1)
                    rec_t = attp.tile([P, 512], F32, tag="rec", bufs=2)
                    nc.vector.reciprocal_approx_fast(rec_t, sum_ps)
                    ctxn_t = attp.tile([P, 512], FP8, tag="ctxn", bufs=2)
                    nc.vector.tensor_mul(ctxn_t, ctx_ps, rec_t)
                    dma_ctx(b, qc, hl, ctxn_t)

                # ---------- phase 1: QKV(b0), standalone ----------
                with (
                    tc.tile_pool(name="qk0", bufs=1) as qk0p,
                    tc.tile_pool(name="v0", bufs=1) as v0p,
                    tc.tile_pool(name="ow0", bufs=1) as ow0p,
                    tc.tile_pool(name="wq", bufs=1) as wqp,
                    tc.tile_pool(name="hid", bufs=2) as hidp,
                    tc.tile_pool(name="qps", bufs=2, space="PSUM") as qps,
                ):
                    qk_sbs[0] = qk0p.tile([P, 2 * HPC, S], FP8, name="qksb0")
                    v_sbs[0] = v0p.tile([P, HPC, 16, P], FP8, name="vsb0")
                    ow_sbs[0] = ow0p.tile([P, HPC, 16, P], FP8, name="owsb0")
                    # wq on sync (HWDGE, ahead of the hid chunks): first
                    # matmul can start as soon as wq + hid(0,0) land (~15us)
                    wq_sb = wqp.tile([P, 16, OSH], FP8)
                    nc.sync.dma_start(
                        wq_sb, wqkvT.rearrange("(ht p) o -> p ht o", p=P)
                    )

                    def qkv_sc(b, sc, vT_sb):
                        """QKV for one 512-wide s-chunk: 6 o-tiles x 8 h-tile
                        pairs (fp8 DoubleRow); call emit(j) for j in
                        range(48).  V^T o-tiles are PE-transposed to V [k, d]
                        and scaled by exp(alibi[k]); ones' tiles built
                        alongside with exp(alibi[k])/16."""
                        hid_t = hidp.tile([P, 16, 512], FP8, tag="hid")
                        nc.sync.dma_start(
                            hid_t,
                            hidT[b * 4 + sc].rearrange("(ht p) q -> p ht q", p=P),
                        )
                        state = {"ps": None}

                        def emit(j):
                            ot, tp = divmod(j, 8)
                            hl, t = divmod(ot, 3)
                            if tp == 0:
                                state["ps"] = qps.tile(
                                    [P, 512], F32, tag="qkvps", bufs=2,
                                    name=f"qps_{b}_{sc}_{ot}",
                                )
                            nc.tensor.matmul(
                                state["ps"],
                                lhsT=wq_sb[:, 2 * tp : 2 * tp + 2, ot * P : (ot + 1) * P],
                                rhs=hid_t[:, 2 * tp : 2 * tp + 2, :],
                                start=(tp == 0),
                                stop=(tp == 7),
                                perf_mode=DR,
                            )
                            if tp == 7:
                                # evacuate on DVE: descale (1/64) + bias in one
                                # dual-scalar op; keeps ScalarE exclusively on Exp
                                dst = (
                                    vT_sb[:, hl, sc * 512 : (sc + 1) * 512]
                                    if t == 2
                                    else qk_sbs[b][:, hl * 2 + t, sc * 512 : (sc + 1) * 512]
                                )
                                nc.vector.tensor_scalar(
                                    out=dst,
                                    in0=state["ps"],
                                    scalar1=1.0 / WSCALE,
                                    scalar2=bq_sb[:, ot : ot + 1],
                                    op0=ALU.mult,
                                    op1=ALU.add,
                                )
                                if t == 2:
                                    # V^T chunk ready: PE-transpose its 4
                                    # k-tiles (psum slots borrowed from the
                                    # qkv pool) and scale rows by exp(alibi)
                                    for kk in range(4):
                                        kt = sc * 4 + kk
                                        acol = (b * HPC + hl) * 16 + kt
                                        vt_ps = qps.tile(
                                            [P, P], BF16, tag="qkvps", bufs=2,
                                            name=f"vt_{b}_{sc}_{hl}_{kk}",
                                        )
                                        nc.tensor.transpose(
                                            vt_ps,
                                            vT_sb[:, hl, kt * P : (kt + 1) * P],
                                            eye_sb,
                                        )
                                        nc.vector.tensor_scalar_mul(
                                            v_sbs[b][:, hl, kt, :],
                                            vt_ps,
                                            ale_sb[:, acol : acol + 1],
                                        )

                        return emit

                    def build_ow(b):
                        """ones' tiles depend only on consts; built on the
                        (otherwise idle) gpsimd engine so they neither
                        serialize the DVE psum-evac chain nor delay it."""
                        for hl in range(HPC):
                            for kt in range(16):
                                acol = (b * HPC + hl) * 16 + kt
                                nc.gpsimd.tensor_scalar_mul(
                                    ow_sbs[b][:, hl, kt, :],
                                    ones_sb,
                                    ale_sb[:, 64 + acol : 64 + acol + 1],
                                )

                    vT0 = vtp.tile([P, HPC, S], BF16, tag="vT", name="vT0")
                    mask_ts = [None] * 4
                    for sc in range(4):
                        emit = qkv_sc(0, sc, vT0)
                        for j in range(48):
                            emit(j)
                        # one mask chunk per s-chunk: spreads the gpsimd DMAs
                        # so chunk 0 lands well before phase 2 needs it
                        mask_ts[sc] = load_mask(sc)
                        if sc == 0:
                            build_ow(0)
                    build_ow(1)

                    # dense weights + residual early: plenty of DMA slack
                    # during phase 2, and it removes the phase-3 entry stall
                    wd_sb = dwp.tile([P, 16, DSH], FP8)
                    nc.gpsimd.dma_start(wd_sb, wdT.rearrange("(ht p) o -> p ht o", p=P))
                    wd2_sb = dwp.tile([P, 16, DSH], FP8)
                    nc.gpsimd.dma_start(wd2_sb, wdT2.rearrange("(ht p) o -> p ht o", p=P))
                    rs_sb = dwp.tile([P, 2, B * S], F32)
                    nc.gpsimd.dma_start(
                        rs_sb[:, :, 0:S],
                        residT[:, 0:S].rearrange("(ot p) s -> p ot s", p=P),
                    )

                    # ---------- phase 2: attention(b0) + QKV(b1) ----------
                    with (
                        tc.tile_pool(name="att", bufs=1) as attp,
                        tc.tile_pool(name="aps", bufs=1, space="PSUM") as aps,
                    ):
                        vT1 = vtp.tile([P, HPC, S], BF16, tag="vT", name="vT1")
                        for qc in range(4):
                            for hl in range(HPC):
                                # 48 QKV(b1) DoubleRow matmuls woven into each
                                # block: 6 MMs per k-tile pair.
                                if hl == 0:
                                    emit = qkv_sc(1, qc, vT1)
                                base = 24 * hl

                                def extra(kp, emit=emit, base=base):
                                    for j in range(3):
                                        emit(base + kp * 3 + j)

                                attn_block(0, qc, hl, mask_ts[qc], aps, attp, extra)
                            if qc == 1:
                                all_gather0(0)
                        all_gather0(1)

                # ---------- phase 3: attention(b1) + dense(b0 + b1 early) --
                with (
                    tc.tile_pool(name="dctx", bufs=4) as dctxp,
                    tc.tile_pool(name="dps", bufs=2, space="PSUM") as dps,
                    tc.tile_pool(name="dout", bufs=3) as doutp,
                ):
                    # b1 residual (b0 half already resident); issued on
                    # gpsimd before any phase-3 collective trigger
                    nc.gpsimd.dma_start(
                        rs_sb[:, :, S : 2 * S],
                        residT[:, S : 2 * S].rearrange("(ot p) s -> p ot s", p=P),
                    )

                    def dense_src(sc, g):
                        """(tensor, col offset, row-pair index, lhsT weights)
                        for h-tile-pair group g of output chunk sc."""
                        if sc < 4:
                            return cc_out0[sc // 2], (sc % 2) * 512, g, wd_sb
                        # per-head gathers: g 0-3 = even heads, 4-7 = odd;
                        # wd2 rows are permuted to match
                        return cc_out1[sc - 4][g // 4], 0, g % 4, wd2_sb

                    def dense_sc(sc):
                        """One 512-wide output column chunk: 8 h-tile-pair
                        groups x 2 o-tiles (fp8 DoubleRow); emit(j) for j in
                        range(16)."""
                        state = {}

                        def emit(j):
                            g, ot = divmod(j, 2)
                            src, col_off, gg, wsb = dense_src(sc, g)
                            if ot == 0:
                                state["ctx"] = dctxp.tile(
                                    [P, 2, 512], FP8, tag="dctx", name="dctx_t"
                                )
                                nc.sync.dma_start(
                                    state["ctx"],
                                    src[
                                        gg * 2 * P : (gg + 1) * 2 * P,
                                        col_off : col_off + 512,
                                    ].rearrange("(a p) q -> p a q", p=P),
                                )
                            if g == 0:
                                state[f"ps{ot}"] = dps.tile(
                                    [P, 512], F32, tag="dps", bufs=2,
                                    name=f"dps_{sc}_{ot}",
                                )
                            nc.tensor.matmul(
                                state[f"ps{ot}"],
                                lhsT=wsb[:, 2 * g : 2 * g + 2, ot * P : (ot + 1) * P],
                                rhs=state["ctx"],
                                start=(g == 0),
                                stop=(g == 7),
                                perf_mode=DR,
                            )
                            if j == 15:
                                for o in range(2):
                                    # descale 1/(64*16) on ACT (Copy shares the
                                    # Exp table: no table reload), residual add
                                    # on DVE
                                    t_t = doutp.tile([P, 512], F32, tag="o")
                                    nc.scalar.activation(
                                        t_t, state[f"ps{o}"], AF.Copy,
                                        scale=1.0 / (WSCALE / OWSCALE),
                                    )
                                    o_t = doutp.tile([P, 512], F32, tag="o")
                                    nc.vector.tensor_add(
                                        o_t,
                                        t_t,
                                        rs_sb[:, o, sc * 512 : (sc + 1) * 512],
                                    )
                                    nc.sync.dma_start(
                                        outT[o * P : (o + 1) * P, sc * 512 : (sc + 1) * 512],
                                        o_t,
                                    )

                        return emit

                    with (
                        tc.tile_pool(name="att1", bufs=1) as attp,
                        tc.tile_pool(name="aps1", bufs=1, space="PSUM") as aps,
                    ):
                        # blocks 0..7 = (qc, hl); dense chunks sc0..sc5
                        # woven into blocks 2..7 (2 MMs per k-tile pair),
                        # leaving each gather time to land before use.
                        DENSE_AT = {2: 0, 3: 1, 4: 2, 5: 3, 6: 4, 7: 5}
                        for qc in range(4):
                            for hl in range(HPC):
                                blk = qc * 2 + hl
                                if blk in DENSE_AT:
                                    emit = dense_sc(DENSE_AT[blk])

                                    def extra(kp, emit=emit):
                                        for j in range(2):
                                            emit(kp * 2 + j)
                                else:
                                    def extra(kp):
                                        pass
                                attn_block(1, qc, hl, mask_ts[qc], aps, attp, extra)
                                # gather this head's ctx immediately
                                all_gather1(qc, hl)

                    # ---------- phase 4: dense tail (last b1 columns) ------
                    for sc in range(6, 8):
                        emit = dense_sc(sc)
                        for j in range(16):
                            emit(j)

    nc.compile()
    return nc


def _prep_in_maps(hidden_states, residual, alibi, attention_mask, w_qkv, b_qkv, w_dense, b_dense):
    f32 = np.float32

    def to_fp8(x):
        return np.clip(x, -240.0, 240.0).astype(NPFP8)

    hs = np.asarray(hidden_states, f32).reshape(B * S, H)
    # packed per 512-wide s-chunk: [B*4, H, 512]
    hidT = to_fp8(
        np.ascontiguousarray(hs.T.reshape(H, B * 4, 512).transpose(1, 0, 2))
    )
    mask_keep = ~np.asarray(attention_mask).reshape(S, S)
    # transposed [k, q], packed per 512-wide q-chunk: [4, S, 512].  The mask
    # carries e^-2 so the total exp shift is 5.5 (max score is 10.24; fp8
    # probs must stay under 240).  A uniform scale cancels in ctx/sum.
    mask01T = np.ascontiguousarray(
        mask_keep.T.reshape(S, 4, 512).transpose(1, 0, 2) * np.exp(-2.0)
    ).astype(NPBF16)
    ones_np = np.ones((P, P), f32).astype(NPBF16)
    al = np.asarray(alibi, f32).reshape(B, NH, S)
    resid = np.asarray(residual, f32).reshape(B * S, H)
    wq = np.asarray(w_qkv, f32)
    bq = np.asarray(b_qkv, f32)
    wd = np.asarray(w_dense, f32)
    bd = np.asarray(b_dense, f32)

    # h-tile row permutation for the per-head b1 gathers: even h-tiles
    # (heads 0,2,..) first, then odd
    perm = [*range(0, 16, 2), *range(1, 16, 2)]

    in_maps = []
    for r in range(NCORES):
        wshard = wq[r * OSH : (r + 1) * OSH]
        bshard = bq[r * OSH : (r + 1) * OSH]
        alcols = []
        for b in range(B):
            for hl in range(HPC):
                alcols.append(np.exp(al[b, HPC * r + hl]).reshape(16, P).T)
        ale = np.concatenate(alcols, axis=1)
        wdsh = wd[r * DSH : (r + 1) * DSH].T  # [H, DSH]
        wdsh2 = wdsh.reshape(16, P, DSH)[perm].reshape(H, DSH)
        in_maps.append(
            {
                "hidT": hidT,
                "wqkvT": to_fp8(np.ascontiguousarray(wshard.T) * WSCALE),
                "bqkv": np.ascontiguousarray(bshard.reshape(6, P).T),
                "mask01T": mask01T,
                "alibi_e": np.ascontiguousarray(
                    np.concatenate([ale, ale * OWSCALE], axis=1)
                ),
                "wdT": to_fp8(np.ascontiguousarray(wdsh) * WSCALE),
                "wdT2": to_fp8(np.ascontiguousarray(wdsh2) * WSCALE),
                "residT": np.ascontiguousarray(resid[:, r * DSH : (r + 1) * DSH].T)
                + bd[r * DSH : (r + 1) * DSH][:, None],
                "ones": ones_np,
                "eye": np.eye(P, dtype=f32).astype(NPBF16),
            }
        )
    return in_maps


if os.environ.get("BASS_LDW_OPT"):
    _orig_run_command = bass_utils.run_command

    def _run_command_ldwopt(argv, **kwargs):
        argv = [
            "--enable-ldw-opt=true" if a == "--enable-ldw-opt=false" else a
            for a in argv
        ]
        return _orig_run_command(argv, **kwargs)

    bass_utils.run_command = _run_command_ldwopt


_NC_CACHE = {}


def run(inputs: dict, trace: bool = False):
    in_maps = _prep_in_maps(**inputs)
    if "nc" not in _NC_CACHE:
        _NC_CACHE["nc"] = build_nc()
    nc = _NC_CACHE["nc"]
    res = bass_utils.run_bass_kernel_spmd(
        nc, in_maps, core_ids=list(range(NCORES)), trace=trace
    )
    out = np.empty((B * S, H), np.float32)
    for r in range(NCORES):
        out[:, r * DSH : (r + 1) * DSH] = res.results[r]["outT"].T
    return out.reshape(B, S, H), res


def kernel(**inputs) -> np.ndarray:
    out, _ = run(inputs, trace=False)
    return out


# revision 61
# speedup vs baseline: 1.2452x; 1.0725x over previous
# Bloom parallel attention block on 8 trn2 NeuronCores, tensor-parallel over
# heads (2 heads per core).  Feature-major layouts, fp8e4 datapath
# (residual/bias/psum fp32; exp intermediates bf16).
#
# Per core r (heads 2r, 2r+1):
#   QKV matmul in fp8e4 DoubleRow (2 h-tiles per PE op, ~2x bf16/instr):
#     hid fp8, weights host-scaled x64 (fp8 subnormal escape), descaled on
#     the DVE evacuation via dual-scalar tensor_scalar (psum*(1/64)+bias).
#     -> Q^T/K^T [d, s] fp8 and V^T [d, s] bf16 per batch in SBUF.
#     inv_norm (1/sqrt(hd)) is applied in the ACT Exp scale, not the weights.
#   V^T is transposed on the PE to V [s, d] and scaled by exp(alibi[k]) on
#   evacuation (fp8); the softmax-denominator matmul weights are
#   exp(alibi[k])/16 broadcast columns ("ones'", fp8, built on gpsimd).
#   This folds alibi in MULTIPLICATIVELY:
#     exp(s + a) * mask = exp(s) * mask * exp(a)
#   The /16 makes rec = 16/sum so ctx^T*rec lands at fp8-friendly scale.
#   attention (per b, head hl, 512-wide q-chunk qc), scores transposed [k, q]:
#     scores^T = K^T_tile.T @ Q^T       fp8, fp32 psum, per k-tile
#     exp(inv_norm*scores - 3.5)        (ACT, 2 k-tiles/op, bf16 out; the
#                                        mask carries e^-2 more: max score
#                                        is 10.24 and fp8 has no saturation)
#     * mask01^T                        (DVE, [P,4,512] op, -> fp8 probs)
#     ctx^T += V'_pair.T @ prob_pair    (PE fp8 DoubleRow, 8 ops/16 k-tiles)
#     sum   += ones'_pair.T @ prob_pair (PE fp8 DoubleRow, denominator)
#     ctx^T *= 16/sum -> fp8 -> DMA to cc chunk
#   Pipelining: QKV(b1) matmuls are interleaved into attention(b0) k-loops,
#   dense matmuls into attention(b1) k-loops, so the PE never idles.  The
#   mask stays fully resident (shared by both batches); the sync queue
#   carries only wq/hid/ctxn/dctx/out, gpsimd carries collectives + cold
#   loads (HWDGE DMAs occupy their queue for the whole transfer).  ctx is
#   AllGathered in fp8: b0 in 2 column chunks, b1 in 8 per-(qc,head)
#   chunks so the final gather is small and the tail drains fast.
#   dense: fp8 DoubleRow (wd host-scaled x64; ctx fp8 carries x16; wdT2 is
#     row-permuted even/odd to match the per-head b1 gather layout):
#     out^T[o_local, s] = wdT_tile.T @ ctx^T_full; evac = ACT Copy
#     (psum/1024) + DVE add of (residual^T + b_dense).
#     (column-parallel => no all-reduce; host concatenates output slices)
import os
import sys

import numpy as np

if "/opt/trn_rl_repo" not in sys.path:
    sys.path.insert(0, "/opt/trn_rl_repo")

import ml_dtypes

import concourse.bass as bass
import concourse.mybir as mybir
import concourse.tile as tile
from concourse import bacc, bass_utils

B, S, H, NH = 2, 2048, 2048, 16
HD = H // NH            # 128
NCORES = 8
HPC = NH // NCORES      # heads per core = 2
OSH = 3 * H // NCORES   # qkv output rows per core = 768
DSH = H // NCORES       # dense output cols per core = 256
P = 128
F32 = mybir.dt.float32
BF16 = mybir.dt.bfloat16
FP8 = mybir.dt.float8e4
AF = mybir.ActivationFunctionType
ALU = mybir.AluOpType
DR = mybir.MatmulPerfMode.DoubleRow
NPBF16 = ml_dtypes.bfloat16
NPFP8 = ml_dtypes.float8_e4m3

WSCALE = 64.0           # host scale on wq/wd to escape fp8 subnormals
OWSCALE = 1.0 / 16.0    # ones' scale => ctx fp8 carries x16
INV_NORM = 1.0 / np.sqrt(HD)


def build_nc():
    nc = bacc.Bacc(
        "TRN2",
        target_bir_lowering=False,
        debug=False,
        num_devices=NCORES,
    )

    # hidT packed per 512-wide s-chunk (contiguous 1MB per chunk => long DMA
    # descriptors); mask01T packed per q-chunk likewise
    hidT = nc.dram_tensor("hidT", [B * 4, H, 512], FP8, kind="ExternalInput").ap()
    wqkvT = nc.dram_tensor("wqkvT", [H, OSH], FP8, kind="ExternalInput").ap()
    bqkv = nc.dram_tensor("bqkv", [P, 6], F32, kind="ExternalInput").ap()
    mask01T = nc.dram_tensor("mask01T", [4, S, 512], BF16, kind="ExternalInput").ap()
    # cols 0-63: exp(alibi) (V' scale); cols 64-127: exp(alibi)/16 (ones')
    alibi_e = nc.dram_tensor("alibi_e", [P, 4 * HPC * 16], F32, kind="ExternalInput").ap()
    wdT = nc.dram_tensor("wdT", [H, DSH], FP8, kind="ExternalInput").ap()
    # h-tile rows permuted [0,2,..,14,1,3,..,15] for the per-head b1 gathers
    wdT2 = nc.dram_tensor("wdT2", [H, DSH], FP8, kind="ExternalInput").ap()
    residT = nc.dram_tensor("residT", [DSH, B * S], F32, kind="ExternalInput").ap()
    ones = nc.dram_tensor("ones", [P, P], BF16, kind="ExternalInput").ap()
    eye = nc.dram_tensor("eye", [P, P], BF16, kind="ExternalInput").ap()
    outT = nc.dram_tensor("outT", [DSH, B * S], F32, kind="ExternalOutput").ap()

    with tile.TileContext(nc) as tc:
        ccg = [list(range(NCORES))]
        with (
            tc.tile_pool(name="const", bufs=1) as constp,
            tc.tile_pool(name="dram", bufs=1, space="DRAM") as dramp,
        ):
            bq_sb = constp.tile([P, 6], F32)
            nc.gpsimd.dma_start(bq_sb, bqkv)
            # shared bias column for the exps: passing a float bias makes
            # bass materialize a const AP per activation call (~25us of DVE
            # setup for 128 exps).  The -3.5 shift keeps exp outputs under
            # fp8e4's 240 max normal (measured score max ~7.8 sigma, and
            # the fp8 cast does NOT saturate: overflow becomes inf); the
            # shift cancels in the ctx/sum ratio.
            shift_col = constp.tile([P, 1], F32)
            nc.vector.memset(shift_col, -3.5)
            ale_sb = constp.tile([P, 4 * HPC * 16], F32)
            nc.gpsimd.dma_start(ale_sb, alibi_e)
            ones_sb = constp.tile(
                [P, P], BF16,
                name="ones_sb_ldw" if os.environ.get("BASS_LDW_OPT") else "ones_sb",
            )
            nc.gpsimd.dma_start(ones_sb, ones)
            eye_sb = constp.tile([P, P], BF16)
            nc.gpsimd.dma_start(eye_sb, eye)

            # ctx gather chunks (fp8): b0 in 4 per-qc chunks [2 heads x 512]
            # (gathered as soon as each q-chunk completes in phase 2),
            # b1 in 8 per-(qc, head) chunks [1 head x 512] so the last gather
            # is tiny and the pipeline drains quickly at the tail.
            cc_in0 = [
                dramp.tile([HPC * HD, S // 2], FP8, name=f"cc_in0{i}")
                for i in range(2)
            ]
            cc_out0 = [
                dramp.tile([NCORES * HPC * HD, S // 2], FP8, addr_space="Shared",
                           name=f"cc_out0{i}")
                for i in range(2)
            ]
            cc_in1 = [
                [dramp.tile([HD, 512], FP8, name=f"cc_in1{qc}{hl}") for hl in range(HPC)]
                for qc in range(4)
            ]
            cc_out1 = [
                [
                    dramp.tile([NCORES * HD, 512], FP8, addr_space="Shared",
                               name=f"cc_out1{qc}{hl}")
                    for hl in range(HPC)
                ]
                for qc in range(4)
            ]

            def dma_ctx(b, qc, hl, ctxn_t):
                if b == 0:
                    chunk, qq = divmod(qc, 2)
                    nc.sync.dma_start(
                        cc_in0[chunk][hl * P : (hl + 1) * P, qq * 512 : (qq + 1) * 512],
                        ctxn_t,
                    )
                else:
                    nc.sync.dma_start(cc_in1[qc][hl], ctxn_t)

            def all_gather0(chunk):
                nc.gpsimd.collective_compute(
                    "AllGather", ALU.bypass, replica_groups=ccg,
                    ins=[cc_in0[chunk].opt()], outs=[cc_out0[chunk].opt()],
                )

            def all_gather1(qc, hl):
                nc.gpsimd.collective_compute(
                    "AllGather", ALU.bypass, replica_groups=ccg,
                    ins=[cc_in1[qc][hl].opt()], outs=[cc_out1[qc][hl].opt()],
                )

            with (
                tc.tile_pool(name="mask", bufs=4) as maskp,
                tc.tile_pool(name="qk1", bufs=1) as qk1p,
                tc.tile_pool(name="vt", bufs=1) as vtp,
                tc.tile_pool(name="v1", bufs=1) as v1p,
                tc.tile_pool(name="ow1", bufs=1) as ow1p,
                tc.tile_pool(name="dw", bufs=1) as dwp,
            ):
                qk_sbs = [None, qk1p.tile([P, 2 * HPC, S], FP8, name="qksb1")]
                v_sbs = [None, v1p.tile([P, HPC, 16, P], FP8, name="vsb1")]
                ow_sbs = [None, ow1p.tile([P, HPC, 16, P], FP8, name="owsb1")]

                def load_mask(qc):
                    """All 4 mask q-chunks stay resident (the mask is shared
                    by both batches, so phases 2 and 3 reuse the same tiles
                    and NO mask DMA competes with compute).  Loaded on
                    gpsimd at phase-1 end, before any collective trigger
                    can block that queue."""
                    m = maskp.tile([P, 16, 512], BF16, tag="mask")
                    nc.gpsimd.dma_start(
                        m,
                        mask01T[qc].rearrange("(kt p) q -> p kt q", p=P),
                    )
                    return m

                def attn_block(b, qc, hl, mask_sb, aps, attp, extra_mm):
                    """Attention for (b, head hl, q-chunk qc).  k-tiles are
                    processed in groups of 4 (2 score-psum pairs, 1 wide mask
                    mul); extra_mm(kp) for kp in 0..7 emits independent
                    matmuls to keep the PE busy while ACT/DVE run."""
                    qk = qk_sbs[b]
                    ctx_ps = aps.tile([P, 512], F32, tag="ctx", bufs=1)
                    sum_ps = aps.tile([P, 512], F32, tag="sum", bufs=1)
                    for kg in range(4):
                        kt0 = 4 * kg
                        exp_t = attp.tile([P, 4, 512], BF16, tag="exp", bufs=3)
                        for u2 in range(2):
                            s_ps = aps.tile([P, 1024], F32, tag="sco", bufs=2)
                            for u in range(2):
                                kt = kt0 + 2 * u2 + u
                                nc.tensor.matmul(
                                    s_ps[:, u * 512 : (u + 1) * 512],
                                    lhsT=qk[:, hl * 2 + 1, kt * P : (kt + 1) * P],
                                    rhs=qk[:, hl * 2, qc * 512 : (qc + 1) * 512],
                                    start=True,
                                    stop=True,
                                )
                            nc.scalar.activation(
                                exp_t[:, 2 * u2 : 2 * u2 + 2, :],
                                s_ps.rearrange("p (u q) -> p u q", u=2),
                                AF.Exp,
                                bias=shift_col[:, 0:1],
                                scale=float(INV_NORM),
                            )
                        prob_t = attp.tile([P, 4, 512], FP8, tag="prob", bufs=3)
                        nc.vector.tensor_mul(
                            prob_t,
                            exp_t,
                            mask_sb[:, kt0 : kt0 + 4, :],
                        )
                        for u2 in range(2):
                            kt = kt0 + 2 * u2
                            ph = prob_t[:, 2 * u2 : 2 * u2 + 2, :]
                            nc.tensor.matmul(
                                ctx_ps,
                                lhsT=v_sbs[b][:, hl, kt : kt + 2, :],
                                rhs=ph,
                                start=(kt == 0),
                                stop=(kt == 14),
                                perf_mode=DR,
                            )
                            nc.tensor.matmul(
                                sum_ps,
                                lhsT=ow_sbs[b][:, hl, kt : kt + 2, :],
                                rhs=ph,
                                start=(kt == 0),
                                stop=(kt == 14),
                                perf_mode=DR,
                            )
                        extra_mm(2 * kg)
                        extra_mm(2 * kg + 1)
                    rec_t = attp.tile([P, 512], F32, tag="rec", bufs=2)
                    nc.vector.reciprocal_approx_fast(rec_t, sum_ps)
                    ctxn_t = attp.tile([P, 512], FP8, tag="ctxn", bufs=2)
                    nc.vector.tensor_mul(ctxn_t, ctx_ps, rec_t)
                    dma_ctx(b, qc, hl, ctxn_t)

                # ---------- phase 1: QKV(b0), standalone ----------
                with (
                    tc.tile_pool(name="qk0", bufs=1) as qk0p,
                    tc.tile_pool(name="v0", bufs=1) as v0p,
                    tc.tile_pool(name="ow0", bufs=1) as ow0p,
                    tc.tile_pool(name="wq", bufs=1) as wqp,
                    tc.tile_pool(name="hid", bufs=2) as hidp,
                    tc.tile_pool(name="qps", bufs=2, space="PSUM") as qps,
                ):
                    qk_sbs[0] = qk0p.tile([P, 2 * HPC, S], FP8, name="qksb0")
                    v_sbs[0] = v0p.tile([P, HPC, 16, P], FP8, name="vsb0")
                    ow_sbs[0] = ow0p.tile([P, HPC, 16, P], FP8, name="owsb0")
                    # wq on sync (HWDGE, ahead of the hid chunks): first
                    # matmul can start as soon as wq + hid(0,0) land (~15us)
                    wq_sb = wqp.tile([P, 16, OSH], FP8)
                    nc.sync.dma_start(
                        wq_sb, wqkvT.rearrange("(ht p) o -> p ht o", p=P)
                    )

                    def qkv_sc(b, sc, vT_sb):
                        """QKV for one 512-wide s-chunk: 6 o-tiles x 8 h-tile
                        pairs (fp8 DoubleRow); call emit(j) for j in
                        range(48).  V^T o-tiles are PE-transposed to V [k, d]
                        and scaled by exp(alibi[k]); ones' tiles built
                        alongside with exp(alibi[k])/16."""
                        hid_t = hidp.tile([P, 16, 512], FP8, tag="hid")
                        nc.sync.dma_start(
                            hid_t,
                            hidT[b * 4 + sc].rearrange("(ht p) q -> p ht q", p=P),
                        )
                        state = {"ps": None}

                        def emit(j):
                            ot, tp = divmod(j, 8)
                            hl, t = divmod(ot, 3)
                            if tp == 0:
                                state["ps"] = qps.tile(
                                    [P, 512], F32, tag="qkvps", bufs=2,
                                    name=f"qps_{b}_{sc}_{ot}",
                                )
                            nc.tensor.matmul(
                                state["ps"],
                                lhsT=wq_sb[:, 2 * tp : 2 * tp + 2, ot * P : (ot + 1) * P],
                                rhs=hid_t[:, 2 * tp : 2 * tp + 2, :],
                                start=(tp == 0),
                                stop=(tp == 7),
                                perf_mode=DR,
                            )
                            if tp == 7:
                                # evacuate on ACT (Identity shares the Exp
                                # table): out = psum/64 + bias.  The DVE is
                                # the block-level bottleneck in phase 2
                                # (mask muls), ACT has slack.
                                dst = (
                                    vT_sb[:, hl, sc * 512 : (sc + 1) * 512]
                                    if t == 2
                                    else qk_sbs[b][:, hl * 2 + t, sc * 512 : (sc + 1) * 512]
                                )
                                nc.scalar.activation(
                                    dst,
                                    state["ps"],
                                    AF.Identity,
                                    bias=bq_sb[:, ot : ot + 1],
                                    scale=1.0 / WSCALE,
                                )
                                if t == 2:
                                    # V^T chunk ready: PE-transpose its 4
                                    # k-tiles (psum slots borrowed from the
                                    # qkv pool) and scale rows by exp(alibi)
                                    for kk in range(4):
                                        kt = sc * 4 + kk
                                        acol = (b * HPC + hl) * 16 + kt
                                        vt_ps = qps.tile(
                                            [P, P], BF16, tag="qkvps", bufs=2,
                                            name=f"vt_{b}_{sc}_{hl}_{kk}",
                                        )
                                        nc.tensor.transpose(
                                            vt_ps,
                                            vT_sb[:, hl, kt * P : (kt + 1) * P],
                                            eye_sb,
                                        )
                                        nc.vector.tensor_scalar_mul(
                                            v_sbs[b][:, hl, kt, :],
                                            vt_ps,
                                            ale_sb[:, acol : acol + 1],
                                        )

                        return emit

                    def build_ow(b):
                        """ones' tiles depend only on consts; built on the
                        (otherwise idle) gpsimd engine so they neither
                        serialize the DVE psum-evac chain nor delay it."""
                        for hl in range(HPC):
                            for kt in range(16):
                                acol = (b * HPC + hl) * 16 + kt
                                nc.gpsimd.tensor_scalar_mul(
                                    ow_sbs[b][:, hl, kt, :],
                                    ones_sb,
                                    ale_sb[:, 64 + acol : 64 + acol + 1],
                                )

                    vT0 = vtp.tile([P, HPC, S], BF16, tag="vT", name="vT0")
                    mask_ts = [None] * 4
                    for sc in range(4):
                        emit = qkv_sc(0, sc, vT0)
                        for j in range(48):
                            emit(j)
                        # one mask chunk per s-chunk: spreads the gpsimd DMAs
                        # so chunk 0 lands well before phase 2 needs it
                        mask_ts[sc] = load_mask(sc)
                        if sc == 0:
                            build_ow(0)
                    build_ow(1)

                    # dense weights + residual early: plenty of DMA slack
                    # during phase 2, and it removes the phase-3 entry stall
                    wd_sb = dwp.tile([P, 16, DSH], FP8)
                    nc.gpsimd.dma_start(wd_sb, wdT.rearrange("(ht p) o -> p ht o", p=P))
                    wd2_sb = dwp.tile([P, 16, DSH], FP8)
                    nc.gpsimd.dma_start(wd2_sb, wdT2.rearrange("(ht p) o -> p ht o", p=P))
                    rs_sb = dwp.tile([P, 2, B * S], F32)
                    nc.gpsimd.dma_start(
                        rs_sb[:, :, 0:S],
                        residT[:, 0:S].rearrange("(ot p) s -> p ot s", p=P),
                    )

                    # ---------- phase 2: attention(b0) + QKV(b1) ----------
                    with (
                        tc.tile_pool(name="att", bufs=1) as attp,
                        tc.tile_pool(name="aps", bufs=1, space="PSUM") as aps,
                    ):
                        vT1 = vtp.tile([P, HPC, S], BF16, tag="vT", name="vT1")
                        for qc in range(4):
                            for hl in range(HPC):
                                # 48 QKV(b1) DoubleRow matmuls woven into each
                                # block: 6 MMs per k-tile pair.
                                if hl == 0:
                                    emit = qkv_sc(1, qc, vT1)
                                base = 24 * hl

                                def extra(kp, emit=emit, base=base):
                                    for j in range(3):
                                        emit(base + kp * 3 + j)

                                attn_block(0, qc, hl, mask_ts[qc], aps, attp, extra)
                            if qc == 1:
                                all_gather0(0)
                        all_gather0(1)

                # ---------- phase 3: attention(b1) + dense(b0 + b1 early) --
                with (
                    tc.tile_pool(name="dctx", bufs=4) as dctxp,
                    tc.tile_pool(name="dps", bufs=2, space="PSUM") as dps,
                    tc.tile_pool(name="dout", bufs=3) as doutp,
                ):
                    # b1 residual (b0 half already resident); issued on
                    # gpsimd before any phase-3 collective trigger
                    nc.gpsimd.dma_start(
                        rs_sb[:, :, S : 2 * S],
                        residT[:, S : 2 * S].rearrange("(ot p) s -> p ot s", p=P),
                    )

                    def dense_src(sc, g):
                        """(tensor, col offset, row-pair index, lhsT weights)
                        for h-tile-pair group g of output chunk sc."""
                        if sc < 4:
                            return cc_out0[sc // 2], (sc % 2) * 512, g, wd_sb
                        # per-head gathers: g 0-3 = even heads, 4-7 = odd;
                        # wd2 rows are permuted to match
                        return cc_out1[sc - 4][g // 4], 0, g % 4, wd2_sb

                    def dense_sc(sc):
                        """One 512-wide output column chunk: 8 h-tile-pair
                        groups x 2 o-tiles (fp8 DoubleRow); emit(j) for j in
                        range(16)."""
                        state = {}

                        def emit(j):
                            g, ot = divmod(j, 2)
                            src, col_off, gg, wsb = dense_src(sc, g)
                            if ot == 0:
                                state["ctx"] = dctxp.tile(
                                    [P, 2, 512], FP8, tag="dctx", name="dctx_t"
                                )
                                nc.sync.dma_start(
                                    state["ctx"],
                                    src[
                                        gg * 2 * P : (gg + 1) * 2 * P,
                                        col_off : col_off + 512,
                                    ].rearrange("(a p) q -> p a q", p=P),
                                )
                            if g == 0:
                                state[f"ps{ot}"] = dps.tile(
                                    [P, 512], F32, tag="dps", bufs=2,
                                    name=f"dps_{sc}_{ot}",
                                )
                            nc.tensor.matmul(
                                state[f"ps{ot}"],
                                lhsT=wsb[:, 2 * g : 2 * g + 2, ot * P : (ot + 1) * P],
                                rhs=state["ctx"],
                                start=(g == 0),
                                stop=(g == 7),
                                perf_mode=DR,
                            )
                            if j == 15:
                                for o in range(2):
                                    # descale 1/(64*16) on ACT (Copy shares the
                                    # Exp table: no table reload), residual add
                                    # on DVE
                                    t_t = doutp.tile([P, 512], F32, tag="o")
                                    nc.scalar.activation(
                                        t_t, state[f"ps{o}"], AF.Copy,
                                        scale=1.0 / (WSCALE / OWSCALE),
                                    )
                                    o_t = doutp.tile([P, 512], F32, tag="o")
                                    nc.vector.tensor_add(
                                        o_t,
                                        t_t,
                                        rs_sb[:, o, sc * 512 : (sc + 1) * 512],
                                    )
                                    nc.sync.dma_start(
                                        outT[o * P : (o + 1) * P, sc * 512 : (sc + 1) * 512],
                                        o_t,
                                    )

                        return emit

                    with (
                        tc.tile_pool(name="att1", bufs=1) as attp,
                        tc.tile_pool(name="aps1", bufs=1, space="PSUM") as aps,
                    ):
                        # blocks 0..7 = (qc, hl); dense chunks sc0..sc5
                        # woven into blocks 2..7 (2 MMs per k-tile pair),
                        # leaving each gather time to land before use.
                        DENSE_AT = {2: 0, 3: 1, 4: 2, 5: 3, 6: 4, 7: 5}
                        for qc in range(4):
                            for hl in range(HPC):
                                blk = qc * 2 + hl
                                if blk in DENSE_AT:
                                    emit = dense_sc(DENSE_AT[blk])

                                    def extra(kp, emit=emit):
                                        for j in range(2):
                                            emit(kp * 2 + j)
                                else:
                                    def extra(kp):
                                        pass
                                attn_block(1, qc, hl, mask_ts[qc], aps, attp, extra)
                                # gather this head's ctx immediately
                                all_gather1(qc, hl)

                    # ---------- phase 4: dense tail (last b1 columns) ------
                    for sc in range(6, 8):
                        emit = dense_sc(sc)
                        for j in range(16):
                            emit(j)

    nc.compile()
    return nc


def _prep_in_maps(hidden_states, residual, alibi, attention_mask, w_qkv, b_qkv, w_dense, b_dense):
    f32 = np.float32

    def to_fp8(x):
        return np.clip(x, -240.0, 240.0).astype(NPFP8)

    hs = np.asarray(hidden_states, f32).reshape(B * S, H)
    # packed per 512-wide s-chunk: [B*4, H, 512]
    hidT = to_fp8(
        np.ascontiguousarray(hs.T.reshape(H, B * 4, 512).transpose(1, 0, 2))
    )
    mask_keep = ~np.asarray(attention_mask).reshape(S, S)
    # transposed [k, q], packed per 512-wide q-chunk: [4, S, 512].  The mask
    # carries e^-2 so the total exp shift is 5.5 (max score is 10.24; fp8
    # probs must stay under 240).  A uniform scale cancels in ctx/sum.
    mask01T = np.ascontiguousarray(
        mask_keep.T.reshape(S, 4, 512).transpose(1, 0, 2) * np.exp(-2.0)
    ).astype(NPBF16)
    ones_np = np.ones((P, P), f32).astype(NPBF16)
    al = np.asarray(alibi, f32).reshape(B, NH, S)
    resid = np.asarray(residual, f32).reshape(B * S, H)
    wq = np.asarray(w_qkv, f32)
    bq = np.asarray(b_qkv, f32)
    wd = np.asarray(w_dense, f32)
    bd = np.asarray(b_dense, f32)

    # h-tile row permutation for the per-head b1 gathers: even h-tiles
    # (heads 0,2,..) first, then odd
    perm = [*range(0, 16, 2), *range(1, 16, 2)]

    in_maps = []
    for r in range(NCORES):
        wshard = wq[r * OSH : (r + 1) * OSH]
        bshard = bq[r * OSH : (r + 1) * OSH]
        alcols = []
        for b in range(B):
            for hl in range(HPC):
                alcols.append(np.exp(al[b, HPC * r + hl]).reshape(16, P).T)
        ale = np.concatenate(alcols, axis=1)
        wdsh = wd[r * DSH : (r + 1) * DSH].T  # [H, DSH]
        wdsh2 = wdsh.reshape(16, P, DSH)[perm].reshape(H, DSH)
        in_maps.append(
            {
                "hidT": hidT,
                "wqkvT": to_fp8(np.ascontiguousarray(wshard.T) * WSCALE),
                "bqkv": np.ascontiguousarray(bshard.reshape(6, P).T),
                "mask01T": mask01T,
                "alibi_e": np.ascontiguousarray(
                    np.concatenate([ale, ale * OWSCALE], axis=1)
                ),
                "wdT": to_fp8(np.ascontiguousarray(wdsh) * WSCALE),
                "wdT2": to_fp8(np.ascontiguousarray(wdsh2) * WSCALE),
                "residT": np.ascontiguousarray(resid[:, r * DSH : (r + 1) * DSH].T)
                + bd[r * DSH : (r + 1) * DSH][:, None],
                "ones": ones_np,
                "eye": np.eye(P, dtype=f32).astype(NPBF16),
            }
        )
    return in_maps


if os.environ.get("BASS_LDW_OPT"):
    _orig_run_command = bass_utils.run_command

    def _run_command_ldwopt(argv, **kwargs):
        argv = [
            "--enable-ldw-opt=true" if a == "--enable-ldw-opt=false" else a
            for a in argv
        ]
        return _orig_run_command(argv, **kwargs)

    bass_utils.run_command = _run_command_ldwopt


_NC_CACHE = {}


def run(inputs: dict, trace: bool = False):
    in_maps = _prep_in_maps(**inputs)
    if "nc" not in _NC_CACHE:
        _NC_CACHE["nc"] = build_nc()
    nc = _NC_CACHE["nc"]
    res = bass_utils.run_bass_kernel_spmd(
        nc, in_maps, core_ids=list(range(NCORES)), trace=trace
    )
    out = np.empty((B * S, H), np.float32)
    for r in range(NCORES):
        out[:, r * DSH : (r + 1) * DSH] = res.results[r]["outT"].T
    return out.reshape(B, S, H), res


def kernel(**inputs) -> np.ndarray:
    out, _ = run(inputs, trace=False)
    return out
